# revision 2
# baseline (speedup 1.0000x reference)
"""GPT-style transformer forward on 8 Trainium2 NeuronCores.

Sharding: data-parallel over batch (2 groups of 4 cores), tensor-parallel
within each group (heads / FFN hidden / vocab columns split 4 ways).
Device activations are feature-major [feature, token] so all matmuls run
without transposes. Two bf16 AllReduces per layer (attention out, FFN out),
one tiny f32 AllReduce for the log-softmax denominator.
"""

import os
from contextlib import ExitStack

import numpy as np
import ml_dtypes

import concourse.bass as bass
import concourse.mybir as mybir
import concourse.tile as tile
from concourse.bass_utils import run_bass_kernel_spmd
from concourse.vector_clock import ScopedClock


def _drain_and_barrier(self, tick_clock, wait_clock):
    """The walrus build here encodes Drain/NoOp as TPB_CTRL with at most one
    sync-wait slot; Tile's stock tail attaches all outstanding waits to the
    Drain and fails codegen. Split the waits one-per-NOP instead."""
    nop_inst = self.nc.sync.nop(nofuse=True)
    wait_clock.add_sem_waits(nop_inst.ins, ScopedClock({None: tick_clock.global_clock}))
    si = nop_inst.ins.sync_info
    if si is not None and len(si.on_wait) > 1:
        waits = list(si.on_wait)
        nop_inst.ins.sync_info = mybir.SyncInfo(on_wait=waits[:1], on_update=list(si.on_update))
        for w in waits[1:]:
            n2 = self.nc.sync.nop(nofuse=True)
            n2.ins.sync_info = mybir.SyncInfo(on_wait=[w], on_update=[])
    self.nc.sync.drain()
    self.nc.all_engine_barrier()
    assert self.sems is not None
    popped = self.nc._tile_sem_poison_stack.pop()
    assert popped is self._sem_poison
    self.nc.clear_and_free_semaphores(list(self.sems.allocated().values()))
    self.nc.all_engine_barrier()


tile.TileContext._drain_and_barrier = _drain_and_barrier

_MAX_WAITS = 1  # this walrus build caps sync-waits per instruction


def split_sync_waits(nc):
    """Hoist excess on_wait entries onto same-engine NOPs inserted before the
    instruction (engine queues execute in program order, so semantics hold)."""
    n = 0
    for bb in nc.main_func.blocks:
        insts = bb.instructions
        i = 0
        new_list = []
        for inst in insts:
            si = getattr(inst, "sync_info", None)
            if si is not None and len(si.on_wait) > _MAX_WAITS:
                waits = list(si.on_wait)
                for w in waits[:-_MAX_WAITS]:
                    n += 1
                    new_list.append(mybir.InstNoOp(
                        name=f"{inst.name}-sw{n}",
                        sync_info=mybir.SyncInfo(on_wait=[w], on_update=[]),
                        bass_nofuse=True,
                        engine=inst.engine,
                    ))
                inst.sync_info = mybir.SyncInfo(
                    on_wait=waits[-_MAX_WAITS:], on_update=list(si.on_update)
                )
            new_list.append(inst)
        if len(new_list) != len(insts):
            bb.instructions[:] = new_list
    return n

# Model dims (hardcoded per problem spec)
L_FULL, H, D, V, SMAX = 8, 16, 1024, 32000, 1024
DH = D // H          # 64
FF = 4 * D           # 4096
B, S = 2, 1024
T = S                # tokens per group (one batch element per group)
TP = 4               # tensor-parallel degree within a group
HL = H // TP         # 4 local heads
FFL = FF // TP       # 1024 local FFN cols
VL = V // TP         # 8000 local vocab cols
VLP = 8064           # padded to 63*128
NVM = VLP // 128     # 63 vocab m-tiles
EPS = 1e-5
KT = D // 128        # 8 k-tiles over model dim
NB = T // 512        # 2 token blocks of 512

BF = mybir.dt.bfloat16
F32 = mybir.dt.float32
AF = mybir.ActivationFunctionType
ALU = mybir.AluOpType

RG = [[0, 1, 2, 3], [4, 5, 6, 7]]

N_LAYERS = int(os.environ.get("BASS_GPT_LAYERS", str(L_FULL)))
SKIP_FINAL = os.environ.get("BASS_GPT_SKIP_FINAL", "0") == "1"


def _r2(ap):
    """[ (kt p) n ] -> [p kt n] view of a DRAM 2-D tensor (p=128)."""
    return ap.rearrange("(kt p) n -> p kt n", p=128)


def build_program():
    nc = bass.Bass("TRN2")

    # ---- DRAM parameters (per-core shards) ----
    h0T = nc.declare_dram_parameter("h0T", [D, T], F32, isOutput=False)
    wqkv = nc.declare_dram_parameter("wqkv", [N_LAYERS, D, 3 * HL * DH], BF, isOutput=False)
    bqkv = nc.declare_dram_parameter("bqkv", [N_LAYERS, 3 * HL * DH], F32, isOutput=False)
    wo = nc.declare_dram_parameter("wo", [N_LAYERS, HL * DH, D], BF, isOutput=False)
    bo4 = nc.declare_dram_parameter("bo4", [N_LAYERS, D], F32, isOutput=False)
    ln1g = nc.declare_dram_parameter("ln1g", [N_LAYERS, D], F32, isOutput=False)
    ln1b = nc.declare_dram_parameter("ln1b", [N_LAYERS, D], F32, isOutput=False)
    w1 = nc.declare_dram_parameter("w1", [N_LAYERS, D, FFL], BF, isOutput=False)
    b1 = nc.declare_dram_parameter("b1", [N_LAYERS, FFL], F32, isOutput=False)
    w2 = nc.declare_dram_parameter("w2", [N_LAYERS, FFL, D], BF, isOutput=False)
    b2q = nc.declare_dram_parameter("b2q", [N_LAYERS, D], F32, isOutput=False)
    ln2g = nc.declare_dram_parameter("ln2g", [N_LAYERS, D], F32, isOutput=False)
    ln2b = nc.declare_dram_parameter("ln2b", [N_LAYERS, D], F32, isOutput=False)
    wout = nc.declare_dram_parameter("wout", [D, VLP], BF, isOutput=False)
    bout = nc.declare_dram_parameter("bout", [VLP], F32, isOutput=False)
    out = nc.declare_dram_parameter("out", [VLP, T], F32, isOutput=True)

    with ExitStack() as ctx:
        tc = ctx.enter_context(tile.TileContext(nc))

        const = ctx.enter_context(tc.tile_pool(name="const", bufs=1))
        hpool = ctx.enter_context(tc.tile_pool(name="hpool", bufs=1))
        xpool = ctx.enter_context(tc.tile_pool(name="xpool", bufs=1))
        apool = ctx.enter_context(tc.tile_pool(name="apool", bufs=1))
        epool = ctx.enter_context(tc.tile_pool(name="epool", bufs=2))
        wq_pool = ctx.enter_context(tc.tile_pool(name="wq_pool", bufs=2))
        wch_pool = ctx.enter_context(tc.tile_pool(name="wch_pool", bufs=3))
        bpool = ctx.enter_context(tc.tile_pool(name="bpool", bufs=2))
        spool = ctx.enter_context(tc.tile_pool(name="spool", bufs=2))
        rpool = ctx.enter_context(tc.tile_pool(name="rpool", bufs=1))
        fpool = ctx.enter_context(tc.tile_pool(name="fpool", bufs=1))

        mm_psum = ctx.enter_context(tc.tile_pool(name="mm_psum", bufs=3, space="PSUM"))
        o_psum = ctx.enter_context(tc.tile_pool(name="o_psum", bufs=1, space="PSUM"))
        bc_psum = ctx.enter_context(tc.tile_pool(name="bc_psum", bufs=2, space="PSUM"))
        st_psum = ctx.enter_context(tc.tile_pool(name="st_psum", bufs=1, space="PSUM"))

        dram = ctx.enter_context(tc.tile_pool(name="dram", bufs=2, space="DRAM"))
        dram1 = ctx.enter_context(tc.tile_pool(name="dram1", bufs=1, space="DRAM"))

        # ---- constants ----
        ones_k = const.tile([128, 1], BF)       # lhsT for partition-sum (K=128, M=1)
        nc.vector.memset(ones_k, 1.0)
        ones_m = const.tile([1, 128], F32)      # lhsT for broadcast (K=1, M=128)
        nc.vector.memset(ones_m, 1.0)
        eps_sb = const.tile([1, 1], F32)
        nc.vector.memset(eps_sb, float(D * D * EPS))
        # causal keep-masks: variant j keeps where t1f - t2p - 128*j >= 0
        maskq = const.tile([128, 4, 512], BF)
        nc.gpsimd.memset(maskq, 1.0)
        for j in range(4):
            nc.gpsimd.affine_select(
                out=maskq[:, j, :], in_=maskq[:, j, :],
                compare_op=ALU.is_ge, fill=0.0,
                base=-128 * j, pattern=[[1, 512]], channel_multiplier=-1,
            )

        # ---- persistent activation state ----
        hb = hpool.tile([128, KT, T], BF)       # residual stream (feature-major)
        x1f = xpool.tile([128, KT, T], F32)     # pre-LN accumulator
        qk_sb = apool.tile([128, 2, 2, T], BF)  # [part, q/k, head-pair, t]
        vaug = apool.tile([128, KT, HL, 65], BF)  # token-major V + ones col
        oT = apool.tile([128, 2, T], BF)        # attn head outputs (feature-major)
        f1 = fpool.tile([128, KT, T], BF)       # FFN hidden (local)

        # load h0 (feature-major) and make bf16 copy
        h0f = xpool.tile([128, KT, T], F32, tag="x1f")  # share slot with x1f
        nc.sync.dma_start(h0f, _r2(h0T))
        for kt in range(KT):
            nc.scalar.activation(hb[:, kt, :], h0f[:, kt, :], AF.Copy)

        def layernorm(xf, g_sb, b_sb, kcol):
            """LN over features of xf [128,KT,T] -> writes hb (bf16).
            g_sb/b_sb: [128, KT] per-feature scale/bias columns (col kcol base)."""
            for nb in range(NB):
                tsl = slice(nb * 512, (nb + 1) * 512)
                ps_s1 = st_psum.tile([1, 512], F32, tag="st1")
                ps_s2 = st_psum.tile([1, 512], F32, tag="st2")
                for kt in range(KT):
                    xb = spool.tile([128, 512], BF, tag="xb")
                    nc.scalar.activation(xb, xf[:, kt, tsl], AF.Copy)
                    nc.tensor.matmul(ps_s1, ones_k, xb, start=(kt == 0), stop=(kt == KT - 1))
                    xsq = spool.tile([128, 512], BF, tag="xsq")
                    nc.scalar.activation(xsq, xf[:, kt, tsl], AF.Square)
                    nc.tensor.matmul(ps_s2, ones_k, xsq, start=(kt == 0), stop=(kt == KT - 1))
                # row math: u = s2*D - s1^2 + D^2*eps ; rp = rsqrt(u)
                s1r = rpool.tile([1, 512], F32, tag="s1r")
                nc.vector.tensor_copy(s1r, ps_s1)
                t0 = rpool.tile([1, 512], F32, tag="t0")
                nc.vector.tensor_mul(t0, s1r, s1r)
                t1 = rpool.tile([1, 512], F32, tag="t1")
                nc.vector.tensor_scalar(out=t1, in0=ps_s2, scalar1=float(D), scalar2=None, op0=ALU.mult)
                nc.vector.tensor_sub(t1, t1, t0)
                rp = rpool.tile([1, 512], F32, tag="rp")
                nc.scalar.activation(rp, t1, AF.Sqrt, bias=eps_sb[0:1, 0:1])
                nc.vector.reciprocal(rp, rp)
                a2 = rpool.tile([1, 512], F32, tag="a2")
                nc.vector.tensor_scalar(out=a2, in0=rp, scalar1=float(D), scalar2=None, op0=ALU.mult)
                c2 = rpool.tile([1, 512], F32, tag="c2")
                nc.vector.tensor_mul(c2, s1r, rp)
                nc.vector.tensor_scalar(out=c2, in0=c2, scalar1=-1.0, scalar2=None, op0=ALU.mult)
                ps_a = bc_psum.tile([128, 512], F32, tag="bc")
                nc.tensor.matmul(ps_a, ones_m, a2, start=True, stop=True)
                ps_c = bc_psum.tile([128, 512], F32, tag="bc")
                nc.tensor.matmul(ps_c, ones_m, c2, start=True, stop=True)
                for kt in range(KT):
                    tt = spool.tile([128, 512], F32, tag="lnt")
                    nc.vector.tensor_mul(tt, xf[:, kt, tsl], ps_a)
                    nc.vector.tensor_add(tt, tt, ps_c)
                    nc.vector.tensor_scalar(
                        out=hb[:, kt, tsl], in0=tt,
                        scalar1=g_sb[:, kcol + kt : kcol + kt + 1],
                        scalar2=b_sb[:, kcol + kt : kcol + kt + 1],
                        op0=ALU.mult, op1=ALU.add,
                    )

        for l in range(N_LAYERS):
            # ---- weights/biases for this layer ----
            wqkv_sb = wq_pool.tile([128, KT, 768], BF, tag="wqkv")
            nc.sync.dma_start(wqkv_sb, _r2(wqkv[l]))
            wo_sb = wq_pool.tile([128, 2, D], BF, tag="wo")
            nc.sync.dma_start(wo_sb, _r2(wo[l]))
            bqkv_sb = bpool.tile([128, 6], F32, tag="bqkv")
            nc.sync.dma_start(bqkv_sb, bqkv[l].rearrange("(m p) -> p m", p=128))
            bo4_sb = bpool.tile([128, KT], F32, tag="bo4")
            nc.sync.dma_start(bo4_sb, bo4[l].rearrange("(m p) -> p m", p=128))
            g1_sb = bpool.tile([128, KT], F32, tag="g1")
            nc.sync.dma_start(g1_sb, ln1g[l].rearrange("(m p) -> p m", p=128))
            bb1_sb = bpool.tile([128, KT], F32, tag="bb1")
            nc.sync.dma_start(bb1_sb, ln1b[l].rearrange("(m p) -> p m", p=128))
            b1_sb = bpool.tile([128, KT], F32, tag="b1")
            nc.sync.dma_start(b1_sb, b1[l].rearrange("(m p) -> p m", p=128))
            b2_sb = bpool.tile([128, KT], F32, tag="b2")
            nc.sync.dma_start(b2_sb, b2q[l].rearrange("(m p) -> p m", p=128))
            g2_sb = bpool.tile([128, KT], F32, tag="g2")
            nc.sync.dma_start(g2_sb, ln2g[l].rearrange("(m p) -> p m", p=128))
            bb2_sb = bpool.tile([128, KT], F32, tag="bb2")
            nc.sync.dma_start(bb2_sb, ln2b[l].rearrange("(m p) -> p m", p=128))

            # ---- Phase A: QKV ----
            for io in range(2):        # 0=q, 1=k  (feature-major out)
                for mt in range(2):    # head pair
                    mcol = (io * 2 + mt) * 128
                    for nb in range(NB):
                        tsl = slice(nb * 512, (nb + 1) * 512)
                        ps = mm_psum.tile([128, 512], F32, tag="mm")
                        for kt in range(KT):
                            nc.tensor.matmul(
                                ps, wqkv_sb[:, kt, mcol : mcol + 128], hb[:, kt, tsl],
                                start=(kt == 0), stop=(kt == KT - 1),
                            )
                        nc.scalar.activation(
                            qk_sb[:, io, mt, tsl], ps, AF.Identity,
                            bias=bqkv_sb[:, io * 2 + mt : io * 2 + mt + 1],
                        )
            nc.vector.memset(vaug[:, :, :, 64:65], 1.0)
            for tm in range(KT):       # v, token-major
                ps = mm_psum.tile([128, 256], F32, tag="mm")
                for kt in range(KT):
                    nc.tensor.matmul(
                        ps, hb[:, kt, tm * 128 : (tm + 1) * 128], wqkv_sb[:, kt, 512:768],
                        start=(kt == 0), stop=(kt == KT - 1),
                    )
                nc.scalar.activation(
                    vaug[:, tm, :, 0:64],
                    ps.rearrange("p (h e) -> p h e", h=HL), AF.Copy,
                )

            # ---- Phase B: attention ----
            for h in range(HL):
                prow = slice(64 * (h % 2), 64 * (h % 2) + 64)
                hm = h // 2
                for blk in range(NB):
                    t1sl = slice(blk * 512, (blk + 1) * 512)
                    t2max = 4 * (blk + 1)
                    et = epool.tile([128, KT, 512], BF, tag="eT")
                    for t2t in range(t2max):
                        ps = mm_psum.tile([128, 512], F32, tag="mm")
                        nc.tensor.matmul(
                            ps,
                            qk_sb[prow, 1, hm, t2t * 128 : (t2t + 1) * 128],
                            qk_sb[prow, 0, hm, t1sl],
                            start=True, stop=True,
                        )
                        nc.scalar.activation(et[:, t2t, :], ps, AF.Exp, scale=0.125)
                        j = t2t - 4 * blk
                        if j >= 0:
                            nc.vector.tensor_mul(et[:, t2t, :], et[:, t2t, :], maskq[:, j, :])
                    ps_o = o_psum.tile([65, 512], F32, tag="o")
                    for t2t in range(t2max):
                        nc.tensor.matmul(
                            ps_o, vaug[:, t2t, h, :], et[:, t2t, :],
                            start=(t2t == 0), stop=(t2t == t2max - 1),
                        )
                    rec = rpool.tile([1, 512], F32, tag="rec")
                    nc.vector.reciprocal(rec, ps_o[64:65, :])
                    ps_b = bc_psum.tile([128, 512], F32, tag="bc")
                    nc.tensor.matmul(ps_b, ones_m, rec, start=True, stop=True)
                    osb = spool.tile([64, 512], F32, tag="osb")
                    nc.scalar.copy(osb, ps_o[0:64, :])
                    tmp = spool.tile([64, 512], F32, tag="otmp")
                    nc.vector.tensor_mul(tmp, osb, ps_b[0:64, :])
                    nc.scalar.activation(
                        oT[prow, hm, t1sl], tmp, AF.Identity,
                        bias=bqkv_sb[prow, 4 + hm : 5 + hm],
                    )

            # ---- Phase C: attn out-proj + AllReduce ----
            ar_in = dram.tile([D, T], BF, tag="arin")
            for mt in range(KT):
                for nb in range(NB):
                    tsl = slice(nb * 512, (nb + 1) * 512)
                    ps = mm_psum.tile([128, 512], F32, tag="mm")
                    for kt in range(2):
                        nc.tensor.matmul(
                            ps, wo_sb[:, kt, mt * 128 : (mt + 1) * 128], oT[:, kt, tsl],
                            start=(kt == 0), stop=(kt == 1),
                        )
                    ob = spool.tile([128, 512], BF, tag="ob")
                    nc.scalar.activation(ob, ps, AF.Identity, bias=bo4_sb[:, mt : mt + 1])
                    nc.sync.dma_start(ar_in[mt * 128 : (mt + 1) * 128, tsl], ob)
            ar_out = dram.tile([D, T], BF, tag="arout")
            nc.gpsimd.collective_compute(
                "AllReduce", ALU.add, replica_groups=RG,
                ins=[ar_in.opt()], outs=[ar_out.opt()],
            )
            # ---- Phase D: x1 = ar + hb ; LN1 -> hb ----
            for kt in range(KT):
                for nb in range(NB):
                    tsl = slice(nb * 512, (nb + 1) * 512)
                    oar_t = spool.tile([128, 512], BF, tag="oar")
                    nc.sync.dma_start(oar_t, _r2(ar_out)[:, kt, tsl])
                    nc.vector.tensor_add(x1f[:, kt, tsl], oar_t, hb[:, kt, tsl])
            layernorm(x1f, g1_sb, bb1_sb, 0)

            # ---- Phase E: FFN ----
            for mt in range(KT):
                w1_sb = wch_pool.tile([128, KT, 128], BF, tag="wch")
                nc.sync.dma_start(w1_sb, _r2(w1[l])[:, :, mt * 128 : (mt + 1) * 128])
                for nb in range(NB):
                    tsl = slice(nb * 512, (nb + 1) * 512)
                    ps = mm_psum.tile([128, 512], F32, tag="mm")
                    for kt in range(KT):
                        nc.tensor.matmul(
                            ps, w1_sb[:, kt, :], hb[:, kt, tsl],
                            start=(kt == 0), stop=(kt == KT - 1),
                        )
                    nc.scalar.activation(
                        f1[:, mt, tsl], ps, AF.Relu, bias=b1_sb[:, mt : mt + 1]
                    )
            ar2_in = dram.tile([D, T], BF, tag="arin")
            for mt in range(KT):
                w2_sb = wch_pool.tile([128, KT, 128], BF, tag="wch")
                nc.sync.dma_start(w2_sb, _r2(w2[l])[:, :, mt * 128 : (mt + 1) * 128])
                for nb in range(NB):
                    tsl = slice(nb * 512, (nb + 1) * 512)
                    ps = mm_psum.tile([128, 512], F32, tag="mm")
                    for kt in range(KT):
                        nc.tensor.matmul(
                            ps, w2_sb[:, kt, :], f1[:, kt, tsl],
                            start=(kt == 0), stop=(kt == KT - 1),
                        )
                    ob = spool.tile([128, 512], BF, tag="ob")
                    nc.scalar.activation(ob, ps, AF.Identity, bias=b2_sb[:, mt : mt + 1])
                    nc.sync.dma_start(ar2_in[mt * 128 : (mt + 1) * 128, tsl], ob)
            ar2_out = dram.tile([D, T], BF, tag="arout")
            nc.gpsimd.collective_compute(
                "AllReduce", ALU.add, replica_groups=RG,
                ins=[ar2_in.opt()], outs=[ar2_out.opt()],
            )
            # ---- Phase F: x2 = ar + hb ; LN2 -> hb ----
            for kt in range(KT):
                for nb in range(NB):
                    tsl = slice(nb * 512, (nb + 1) * 512)
                    oar_t = spool.tile([128, 512], BF, tag="oar")
                    nc.sync.dma_start(oar_t, _r2(ar2_out)[:, kt, tsl])
                    nc.vector.tensor_add(x1f[:, kt, tsl], oar_t, hb[:, kt, tsl])
            layernorm(x1f, g2_sb, bb2_sb, 0)

        # ---- Phase G: vocab projection + log-softmax ----
        if not SKIP_FINAL:
            bout_sb = const.tile([128, NVM], F32)
            nc.sync.dma_start(bout_sb, bout.rearrange("(m p) -> p m", p=128))
            logits_stage = dram1.tile([VLP, T], BF, tag="lst")
            se_in = dram.tile([NB, 512], F32, tag="sein")
            ps_se = []
            for nb in range(NB):
                tsl = slice(nb * 512, (nb + 1) * 512)
                ps_acc = st_psum.tile([1, 512], F32, tag="st1")
                for vm in range(NVM):
                    wv_sb = wch_pool.tile([128, KT, 128], BF, tag="wch")
                    nc.sync.dma_start(wv_sb, _r2(wout)[:, :, vm * 128 : (vm + 1) * 128])
                    ps = mm_psum.tile([128, 512], F32, tag="mm")
                    for kt in range(KT):
                        nc.tensor.matmul(
                            ps, wv_sb[:, kt, :], hb[:, kt, tsl],
                            start=(kt == 0), stop=(kt == KT - 1),
                        )
                    lb = spool.tile([128, 512], BF, tag="lb")
                    nc.scalar.activation(lb, ps, AF.Identity, bias=bout_sb[:, vm : vm + 1])
                    nc.sync.dma_start(
                        logits_stage[vm * 128 : (vm + 1) * 128, tsl], lb
                    )
                    eb = spool.tile([128, 512], BF, tag="eb")
                    nc.scalar.activation(eb, ps, AF.Exp, bias=bout_sb[:, vm : vm + 1])
                    nc.tensor.matmul(
                        ps_acc, ones_k, eb,
                        start=(vm == 0), stop=(vm == NVM - 1), skip_group_check=True,
                    )
                se_row = rpool.tile([1, 512], F32, tag="serow")
                nc.vector.tensor_copy(se_row, ps_acc)
                nc.sync.dma_start(se_in[nb : nb + 1, :], se_row)
                ps_se.append(ps_acc)
            se_out = dram.tile([NB, 512], F32, tag="seout")
            nc.gpsimd.collective_compute(
                "AllReduce", ALU.add, replica_groups=RG,
                ins=[se_in.opt()], outs=[se_out.opt()],
            )
            se_sb = const.tile([1, NB, 512], F32)
            nc.sync.dma_start(se_sb, se_out[:].rearrange("(o a) b -> o a b", o=1))
            ps_L = []
            for nb in range(NB):
                lr = rpool.tile([1, 512], F32, tag="lr")
                nc.scalar.activation(lr, se_sb[0:1, nb, :], AF.Ln)
                psl = bc_psum.tile([128, 512], F32, tag="bc")
                nc.tensor.matmul(psl, ones_m, lr, start=True, stop=True)
                ps_L.append(psl)
            for nb in range(NB):
                tsl = slice(nb * 512, (nb + 1) * 512)
                for vm in range(NVM):
                    lb2 = spool.tile([128, 512], BF, tag="lb2")
                    nc.sync.dma_start(lb2, logits_stage[vm * 128 : (vm + 1) * 128, tsl])
                    outf = spool.tile([128, 512], F32, tag="outf")
                    nc.vector.tensor_sub(outf, lb2, ps_L[nb])
                    nc.sync.dma_start(out[vm * 128 : (vm + 1) * 128, tsl], outf)
        else:
            # debug: dump hb as f32 into the first D rows of out
            for kt in range(KT):
                dbg = spool.tile([128, T], F32, tag="outf")
                nc.scalar.activation(dbg, hb[:, kt, :], AF.Copy)
                nc.sync.dma_start(out[kt * 128 : (kt + 1) * 128, :], dbg)

    nsplit = split_sync_waits(nc)
    print(f"split_sync_waits: {nsplit} NOPs inserted")
    return nc


def _bf16(a):
    return np.asarray(a, dtype=ml_dtypes.bfloat16)


def make_in_maps(x, tok_emb, pos_emb, wq, bq, wk, bk, wv, bv, wo, bo,
                 ln1_g, ln1_b, w1, b1, w2, b2, ln2_g, ln2_b, w_out, b_out):
    """Shard full inputs -> per-core input maps."""
    LE = wq.shape[0]
    per_r = []
    for r in range(TP):
        hs = slice(HL * r, HL * (r + 1))
        wqkv_r = np.concatenate(
            [
                wq[:, hs].transpose(0, 2, 1, 3).reshape(LE, D, HL * DH),
                wk[:, hs].transpose(0, 2, 1, 3).reshape(LE, D, HL * DH),
                wv[:, hs].transpose(0, 2, 1, 3).reshape(LE, D, HL * DH),
            ],
            axis=2,
        )
        bqkv_r = np.concatenate(
            [bq[:, hs].reshape(LE, -1), bk[:, hs].reshape(LE, -1),
             bv[:, hs].reshape(LE, -1)], axis=1,
        )
        fs = slice(FFL * r, FFL * (r + 1))
        vs = slice(VL * r, VL * (r + 1))
        wout_r = np.zeros((D, VLP), np.float32)
        wout_r[:, :VL] = w_out[:, vs]
        bout_r = np.full((VLP,), -1e30, np.float32)
        bout_r[:VL] = b_out[vs]
        per_r.append(dict(
            wqkv=_bf16(wqkv_r),
            bqkv=np.ascontiguousarray(bqkv_r, np.float32),
            wo=_bf16(wo[:, DH * HL * r : DH * HL * (r + 1), :]),
            bo4=np.ascontiguousarray(bo / TP, np.float32),
            ln1g=np.ascontiguousarray(ln1_g, np.float32),
            ln1b=np.ascontiguousarray(ln1_b, np.float32),
            w1=_bf16(w1[:, :, fs]),
            b1=np.ascontiguousarray(b1[:, fs], np.float32),
            w2=_bf16(w2[:, fs, :]),
            b2q=np.ascontiguousarray(b2 / TP, np.float32),
            ln2g=np.ascontiguousarray(ln2_g, np.float32),
            ln2b=np.ascontiguousarray(ln2_b, np.float32),
            wout=_bf16(wout_r),
            bout=bout_r,
        ))
    in_maps = []
    for c in range(8):
        g, r = c // TP, c % TP
        emb = tok_emb[x[g]] + pos_emb[:S]          # [S, D]
        m = dict(per_r[r])
        m["h0T"] = np.ascontiguousarray(emb.T, np.float32)
        in_maps.append(m)
    return in_maps


_CACHED = {}


def _install_ntff_shim():
    """Provide antenv.axon_hooks.get_axon_ntff_profile_hook via ctypes on
    libaxon_pjrt.so (this container's trn_rl_repo snapshot lacks the module)."""
    import sys
    import types
    import ctypes
    import contextlib

    if "antenv.axon_hooks" in sys.modules:
        return
    try:
        lib = ctypes.CDLL("/opt/axon/libaxon_pjrt.so")
    except OSError:
        return
    lib.axon_start_nrt_profile.restype = ctypes.c_int64
    lib.axon_start_nrt_profile.argtypes = [ctypes.c_char_p, ctypes.c_size_t]
    lib.axon_stop_nrt_profile.restype = ctypes.c_int64
    lib.axon_stop_nrt_profile.argtypes = [ctypes.c_char_p]

    def get_axon_ntff_profile_hook():
        @contextlib.contextmanager
        def hook(neff_dir, trace_model_indices):
            d = str(neff_dir).encode()
            if lib.axon_start_nrt_profile(d, len(d)) != 0:
                yield
                return
            try:
                yield
            finally:
                lib.axon_stop_nrt_profile(d)

        return hook

    mod = types.ModuleType("antenv.axon_hooks")
    mod.get_axon_ntff_profile_hook = get_axon_ntff_profile_hook
    sys.modules["antenv.axon_hooks"] = mod


def kernel(**inputs):
    inputs = {k: np.asarray(v) for k, v in inputs.items()}
    if "nc" not in _CACHED:
        _CACHED["nc"] = build_program()
    nc = _CACHED["nc"]
    in_maps = make_in_maps(**inputs)
    trace = os.environ.get("BASS_GPT_TRACE", "0") == "1"
    if trace:
        _install_ntff_shim()
    res = run_bass_kernel_spmd(
        nc, in_maps, core_ids=list(range(8)), trace=trace,
    )
    if trace:
        print(f"HW exec time: {res.exec_time_ns} ns")
        _CACHED["last_result"] = res
    results = res.results
    full = np.empty((B, S, V), np.float32)
    for c in range(8):
        g, r = c // TP, c % TP
        full[g, :, VL * r : VL * (r + 1)] = results[c]["out"][:VL, :].T
    return full



# revision 19
# speedup vs baseline: 1.4286x; 1.4286x over previous
"""GPT-style transformer forward on 8 Trainium2 NeuronCores.

Sharding: data-parallel over batch (2 groups of 4 cores), tensor-parallel
within each group (heads / FFN hidden / vocab columns split 4 ways).
Device activations are feature-major [feature, token] so all matmuls run
without transposes.

Schedule: per layer the token dim is split into two 512-blocks; each
block's AllReduce is issued while the other block computes (attention,
FFN, or the next layer's QKV), hiding collective latency.  LN gamma/beta
and all linear biases are folded into weights / eviction biases host-side,
so the device residual+LN is: fused residual-scale-add (gpsimd STT) ->
sum/sumsq ones-matmuls -> rsqrt -> two broadcast-apply ops.  Causal
attention computes only unmasked column ranges; diagonal tiles are masked
in-place with affine_select.  The log-softmax denominator is reduced on
device (per-core sum-exp) and the final subtract is folded into the
host-side unshard.
"""

import os
from contextlib import ExitStack

import numpy as np
import ml_dtypes

import concourse.bass as bass
import concourse.mybir as mybir
import concourse.tile as tile
from concourse.bass_utils import run_bass_kernel_spmd
from concourse.vector_clock import ScopedClock


def _drain_and_barrier(self, tick_clock, wait_clock):
    """The walrus build here encodes Drain/NoOp as TPB_CTRL with at most one
    sync-wait slot; Tile's stock tail attaches all outstanding waits to the
    Drain and fails codegen. Split the waits one-per-NOP instead."""
    nop_inst = self.nc.sync.nop(nofuse=True)
    wait_clock.add_sem_waits(nop_inst.ins, ScopedClock({None: tick_clock.global_clock}))
    si = nop_inst.ins.sync_info
    if si is not None and len(si.on_wait) > 1:
        waits = list(si.on_wait)
        nop_inst.ins.sync_info = mybir.SyncInfo(on_wait=waits[:1], on_update=list(si.on_update))
        for w in waits[1:]:
            n2 = self.nc.sync.nop(nofuse=True)
            n2.ins.sync_info = mybir.SyncInfo(on_wait=[w], on_update=[])
    self.nc.sync.drain()
    self.nc.all_engine_barrier()
    assert self.sems is not None
    popped = self.nc._tile_sem_poison_stack.pop()
    assert popped is self._sem_poison
    self.nc.clear_and_free_semaphores(list(self.sems.allocated().values()))
    self.nc.all_engine_barrier()


tile.TileContext._drain_and_barrier = _drain_and_barrier

_MAX_WAITS = 1  # this walrus build caps sync-waits per instruction


def split_sync_waits(nc):
    """Hoist excess on_wait entries onto same-engine NOPs inserted before the
    instruction (engine queues execute in program order, so semantics hold)."""
    n = 0
    for bb in nc.main_func.blocks:
        insts = bb.instructions
        new_list = []
        for inst in insts:
            si = getattr(inst, "sync_info", None)
            if si is not None and len(si.on_wait) > _MAX_WAITS:
                waits = list(si.on_wait)
                for w in waits[:-_MAX_WAITS]:
                    n += 1
                    new_list.append(mybir.InstNoOp(
                        name=f"{inst.name}-sw{n}",
                        sync_info=mybir.SyncInfo(on_wait=[w], on_update=[]),
                        bass_nofuse=True,
                        engine=inst.engine,
                    ))
                inst.sync_info = mybir.SyncInfo(
                    on_wait=waits[-_MAX_WAITS:], on_update=list(si.on_update)
                )
            new_list.append(inst)
        if len(new_list) != len(insts):
            bb.instructions[:] = new_list
    return n


# Model dims (hardcoded per problem spec)
L_FULL, H, D, V, SMAX = 8, 16, 1024, 32000, 1024
DH = D // H          # 64
FF = 4 * D           # 4096
B, S = 2, 1024
T = S                # tokens per group (one batch element per group)
TP = 4               # tensor-parallel degree within a group
HL = H // TP         # 4 local heads
FFL = FF // TP       # 1024 local FFN cols
VL = V // TP         # 8000 local vocab cols
VLP = 8064           # padded to 63*128
NVM = VLP // 128     # 63 vocab m-tiles
EPS = 1e-5
KT = D // 128        # 8 k-tiles over model dim
NB = T // 512        # 2 token blocks of 512

BF = mybir.dt.bfloat16
F32 = mybir.dt.float32
AF = mybir.ActivationFunctionType
ALU = mybir.AluOpType

RG = [[0, 1, 2, 3], [4, 5, 6, 7]]

N_LAYERS = int(os.environ.get("BASS_GPT_LAYERS", str(L_FULL)))
SKIP_FINAL = os.environ.get("BASS_GPT_SKIP_FINAL", "0") == "1"


def _r2(ap):
    """[ (kt p) n ] -> [p kt n] view of a DRAM 2-D tensor (p=128)."""
    return ap.rearrange("(kt p) n -> p kt n", p=128)


def build_program():
    nc = bass.Bass("TRN2")

    # ---- DRAM parameters (per-core shards; gamma/beta/bias folded host-side) ----
    h0T = nc.declare_dram_parameter("h0T", [D, T], BF, isOutput=False)
    wqkv = nc.declare_dram_parameter("wqkv", [N_LAYERS, D, 3 * HL * DH], BF, isOutput=False)
    bqk = nc.declare_dram_parameter("bqk", [N_LAYERS, 4 * 128], F32, isOutput=False)
    wo = nc.declare_dram_parameter("wo", [N_LAYERS, HL * DH, D], BF, isOutput=False)
    evb1 = nc.declare_dram_parameter("evb1", [N_LAYERS, D], F32, isOutput=False)
    evb2 = nc.declare_dram_parameter("evb2", [N_LAYERS, D], F32, isOutput=False)
    resga = nc.declare_dram_parameter("resga", [N_LAYERS, D], F32, isOutput=False)
    resgb = nc.declare_dram_parameter("resgb", [N_LAYERS, D], F32, isOutput=False)
    w1 = nc.declare_dram_parameter("w1", [N_LAYERS, D, FFL], BF, isOutput=False)
    b1p = nc.declare_dram_parameter("b1p", [N_LAYERS, FFL], F32, isOutput=False)
    w2 = nc.declare_dram_parameter("w2", [N_LAYERS, FFL, D], BF, isOutput=False)
    wout = nc.declare_dram_parameter("wout", [D, VLP], BF, isOutput=False)
    bout = nc.declare_dram_parameter("bout", [VLP], F32, isOutput=False)
    out = nc.declare_dram_parameter("out", [VLP, T], F32, isOutput=True)
    sumexp = nc.declare_dram_parameter("sumexp", [NB, 512], F32, isOutput=True)

    with ExitStack() as ctx:
        tc = ctx.enter_context(tile.TileContext(nc))

        const = ctx.enter_context(tc.tile_pool(name="const", bufs=1))
        hpool = ctx.enter_context(tc.tile_pool(name="hpool", bufs=1))
        apool = ctx.enter_context(tc.tile_pool(name="apool", bufs=1))
        epool = ctx.enter_context(tc.tile_pool(name="epool", bufs=1))
        s1pool = ctx.enter_context(tc.tile_pool(name="s1pool", bufs=1))
        wq_pool = ctx.enter_context(tc.tile_pool(name="wq_pool", bufs=2))
        wf_pool = ctx.enter_context(tc.tile_pool(name="wf_pool", bufs=1))
        wch_pool = ctx.enter_context(tc.tile_pool(name="wch_pool", bufs=3))
        bpool = ctx.enter_context(tc.tile_pool(name="bpool", bufs=2))
        spool = ctx.enter_context(tc.tile_pool(name="spool", bufs=2))
        rpool = ctx.enter_context(tc.tile_pool(name="rpool", bufs=2))
        fpool = ctx.enter_context(tc.tile_pool(name="fpool", bufs=1))

        mm_psum = ctx.enter_context(tc.tile_pool(name="mm_psum", bufs=3, space="PSUM"))
        o_psum = ctx.enter_context(tc.tile_pool(name="o_psum", bufs=2, space="PSUM"))
        st_psum = ctx.enter_context(tc.tile_pool(name="st_psum", bufs=1, space="PSUM"))
        bc_psum = ctx.enter_context(tc.tile_pool(name="bc_psum", bufs=1, space="PSUM"))

        dram = ctx.enter_context(tc.tile_pool(name="dram", bufs=3, space="DRAM"))

        # ---- constants ----
        ones_d = const.tile([128, 1], BF)       # partition-sum lhsT, scaled 1/D (LN stats)
        nc.vector.memset(ones_d, 1.0 / D)
        ones_1 = const.tile([128, 1], BF)       # partition-sum lhsT (softmax denominator)
        nc.vector.memset(ones_1, 1.0)
        ones_m = const.tile([1, 128], BF)       # broadcast lhsT (K=1, M=128)
        nc.vector.memset(ones_m, 1.0)
        negones_m = const.tile([1, 128], BF)    # negated broadcast lhsT
        nc.vector.memset(negones_m, -1.0)
        eps_sb = const.tile([1, 1], F32)
        nc.vector.memset(eps_sb, float(EPS))

        # ---- persistent activation state ----
        hb = hpool.tile([128, KT, T], BF)       # residual stream (feature-major, pre-gamma/beta)
        qk_sb = apool.tile([128, 2, 2, T], BF)  # [part, q/k, head-pair, t]
        vaug = apool.tile([128, KT, HL, 65], BF)  # token-major V + ones col
        oT = apool.tile([128, 2, T], BF)        # attn head outputs (feature-major, normalized)
        f1 = fpool.tile([128, KT, T], BF)       # FFN hidden (local)

        nc.sync.dma_start(hb, _r2(h0T))         # h0 straight into the residual stream
        nc.vector.memset(vaug[:, :, :, 64:65], 1.0)

        def ln_block(nb, ar_out, gcol):
            """hb[:, :, tsl] <- normalize(hb * gcol + AR result) (token block nb)."""
            tsl = slice(nb * 512, (nb + 1) * 512)
            arb = s1pool.tile([128, KT, 512], BF, tag="arb")
            nc.sync.dma_start(arb, _r2(ar_out))
            xb = spool.tile([128, KT, 512], BF, tag="xb")
            ps_st = st_psum.tile([33, 512], F32, tag="st")
            for kt in range(KT):
                nc.vector.scalar_tensor_tensor(
                    out=xb[:, kt, :], in0=hb[:, kt, tsl],
                    scalar=gcol[:, kt : kt + 1], in1=arb[:, kt, :],
                    op0=ALU.mult, op1=ALU.add,
                )
                xsq = spool.tile([128, 512], BF, tag="xsq")
                nc.scalar.activation(xsq, xb[:, kt, :], AF.Square)
                nc.tensor.matmul(ps_st[0:1, :], ones_d, xb[:, kt, :],
                                 start=(kt == 0), stop=(kt == KT - 1),
                                 skip_group_check=True)
                nc.tensor.matmul(ps_st[32:33, :], ones_d, xsq,
                                 start=(kt == 0), stop=(kt == KT - 1),
                                 skip_group_check=True)
            # u = m2 - mu^2 ; a = rsqrt(u + eps) = exp(-0.5 ln(u + eps))
            t0 = rpool.tile([1, 512], F32, tag="t0")
            nc.scalar.activation(t0, ps_st[0:1, :], AF.Square)
            u = rpool.tile([1, 512], F32, tag="u")
            nc.vector.tensor_sub(u, ps_st[32:33, :], t0)
            lnr = rpool.tile([1, 512], F32, tag="lnr")
            nc.scalar.activation(lnr, u, AF.Ln, bias=eps_sb[0:1, 0:1])
            a_row = rpool.tile([1, 512], BF, tag="arow")
            nc.scalar.activation(a_row, lnr, AF.Exp, scale=-0.5)
            s1r = rpool.tile([1, 512], BF, tag="s1r")
            nc.vector.tensor_copy(s1r, ps_st[0:1, :])
            # broadcast -mu and a across partitions via K=1 matmuls, stage in SBUF
            ps_nm = bc_psum.tile([128, 512], F32, tag="bc")
            nc.tensor.matmul(ps_nm, negones_m, s1r, start=True, stop=True)
            nmb = spool.tile([128, 512], BF, tag="nmb")
            nc.scalar.activation(nmb, ps_nm, AF.Identity)
            ps_ab = bc_psum.tile([128, 512], F32, tag="bc")
            nc.tensor.matmul(ps_ab, ones_m, a_row, start=True, stop=True)
            ab = spool.tile([128, 512], BF, tag="ab")
            nc.vector.tensor_copy(ab, ps_ab)
            for kt in range(KT):
                tt = spool.tile([128, 512], BF, tag="lnt")
                nc.gpsimd.tensor_add(tt, xb[:, kt, :], nmb)
                nc.gpsimd.tensor_mul(hb[:, kt, tsl], tt, ab)

        def evict(ps, out_ap, col=None, relu=False, eng="scalar"):
            """PSUM -> SBUF eviction with optional per-partition bias / relu.
            (gpsimd cannot read PSUM, so only scalar/vector qualify.)"""
            if eng == "scalar":
                nc.scalar.activation(out_ap, ps, AF.Relu if relu else AF.Identity,
                                     bias=col if col is not None else 0.0)
            else:
                e = nc.vector
                if relu:
                    e.tensor_scalar(out=out_ap, in0=ps,
                                    scalar1=col if col is not None else 0.0,
                                    scalar2=0.0, op0=ALU.add, op1=ALU.max)
                elif col is not None:
                    e.tensor_scalar(out=out_ap, in0=ps, scalar1=col, scalar2=None,
                                    op0=ALU.add)
                else:
                    e.tensor_copy(out_ap, ps)

        RR = ("scalar", "vector")

        def qkv_block(nb, wqkv_sb, bqk_sb):
            tsl = slice(nb * 512, (nb + 1) * 512)
            for io in range(2):        # 0=q, 1=k  (feature-major out)
                for mt in range(2):    # head pair
                    mcol = (io * 2 + mt) * 128
                    ps = mm_psum.tile([128, 512], F32, tag="mm")
                    for kt in range(KT):
                        nc.tensor.matmul(
                            ps, wqkv_sb[:, kt, mcol : mcol + 128], hb[:, kt, tsl],
                            start=(kt == 0), stop=(kt == KT - 1),
                        )
                    evict(ps, qk_sb[:, io, mt, tsl],
                          col=bqk_sb[:, io * 2 + mt : io * 2 + mt + 1],
                          eng=RR[(io * 2 + mt) % 2])
            for tm in range(nb * 4, nb * 4 + 4):   # v, token-major
                ps = mm_psum.tile([128, 512], F32, tag="mm")
                for kt in range(KT):
                    nc.tensor.matmul(
                        ps[:, 0:256], hb[:, kt, tm * 128 : (tm + 1) * 128],
                        wqkv_sb[:, kt, 512:768],
                        start=(kt == 0), stop=(kt == KT - 1),
                    )
                evict(ps[:, 0:256].rearrange("p (h e) -> p h e", h=HL),
                      vaug[:, tm, :, 0:64], eng=RR[tm % 2])

        def attn_block(blk):
            t1base = blk * 512
            t2max = 4 * (blk + 1)
            for h in range(HL):
                prow = slice(64 * (h % 2), 64 * (h % 2) + 64)
                hm = h // 2
                et = epool.tile([128, 8, 512], BF, tag="eT")
                for t2t in range(t2max):
                    a = max(0, 128 * (t2t - 4 * blk))
                    ps = mm_psum.tile([128, 512], F32, tag="mm")
                    nc.tensor.matmul(
                        ps[:, a:],
                        qk_sb[prow, 1, hm, t2t * 128 : (t2t + 1) * 128],
                        qk_sb[prow, 0, hm, t1base + a : t1base + 512],
                        start=True, stop=True,
                    )
                    nc.scalar.activation(et[:, t2t, a:], ps[:, a:], AF.Exp, scale=0.125)
                    j = t2t - 4 * blk
                    if 0 <= j <= 3:
                        # diagonal 128x128 triangle: keep where t1 - t2 >= 0
                        nc.gpsimd.affine_select(
                            out=et[:, t2t, a : a + 128], in_=et[:, t2t, a : a + 128],
                            compare_op=ALU.is_ge, fill=0.0,
                            base=0, pattern=[[1, 128]], channel_multiplier=-1,
                        )
                ps_o = o_psum.tile([65, 512], F32, tag="o")
                for t2t in range(t2max):
                    a = max(0, 128 * (t2t - 4 * blk))
                    nc.tensor.matmul(
                        ps_o[:, a:], vaug[:, t2t, h, :], et[:, t2t, a:],
                        start=(t2t == 0), stop=(t2t == t2max - 1),
                        skip_group_check=True,
                    )
                # 1/den = exp(-ln(den)); broadcast over the 64 head dims via PE
                lnd = rpool.tile([1, 512], F32, tag="lnd")
                nc.scalar.activation(lnd, ps_o[64:65, :], AF.Ln)
                rec = rpool.tile([1, 512], BF, tag="rec")
                nc.scalar.activation(rec, lnd, AF.Exp, scale=-1.0)
                ps_rb = bc_psum.tile([128, 512], F32, tag="bc")
                nc.tensor.matmul(ps_rb[0:64, :], ones_m[:, 0:64], rec,
                                 start=True, stop=True)
                osb = spool.tile([64, 512], BF, tag="osb")
                nc.scalar.activation(osb, ps_o[0:64, :], AF.Identity)
                nc.vector.tensor_mul(oT[prow, hm, t1base : t1base + 512],
                                     osb, ps_rb[0:64, :])

        def outproj_block(nb, wo_sb, ev1_sb):
            tsl = slice(nb * 512, (nb + 1) * 512)
            ar_in = dram.tile([D, 512], BF, tag="arin")
            for mt in range(KT):
                ps = mm_psum.tile([128, 512], F32, tag="mm")
                for kt in range(2):
                    nc.tensor.matmul(
                        ps, wo_sb[:, kt, mt * 128 : (mt + 1) * 128], oT[:, kt, tsl],
                        start=(kt == 0), stop=(kt == 1),
                    )
                ob = spool.tile([128, 512], BF, tag="ob")
                evict(ps, ob, col=ev1_sb[:, mt : mt + 1], eng=RR[mt % 2])
                nc.sync.dma_start(ar_in[mt * 128 : (mt + 1) * 128, :], ob)
            ar_out = dram.tile([D, 512], BF, tag="arout")
            nc.gpsimd.collective_compute(
                "AllReduce", ALU.add, replica_groups=RG,
                ins=[ar_in.opt()], outs=[ar_out.opt()],
            )
            return ar_out

        def ffn_block(nb, w1_sb, w2_sb, b1_sb, ev2_sb):
            tsl = slice(nb * 512, (nb + 1) * 512)
            for mt in range(KT):
                ps = mm_psum.tile([128, 512], F32, tag="mm")
                for kt in range(KT):
                    nc.tensor.matmul(
                        ps, w1_sb[:, kt, mt * 128 : (mt + 1) * 128], hb[:, kt, tsl],
                        start=(kt == 0), stop=(kt == KT - 1),
                    )
                evict(ps, f1[:, mt, tsl], col=b1_sb[:, mt : mt + 1], relu=True,
                      eng=RR[mt % 2])
            ar_in = dram.tile([D, 512], BF, tag="arin")
            for mt in range(KT):
                ps = mm_psum.tile([128, 512], F32, tag="mm")
                for kt in range(KT):
                    nc.tensor.matmul(
                        ps, w2_sb[:, kt, mt * 128 : (mt + 1) * 128], f1[:, kt, tsl],
                        start=(kt == 0), stop=(kt == KT - 1),
                    )
                ob = spool.tile([128, 512], BF, tag="ob")
                evict(ps, ob, col=ev2_sb[:, mt : mt + 1], eng=RR[(mt + 1) % 2])
                nc.sync.dma_start(ar_in[mt * 128 : (mt + 1) * 128, :], ob)
            ar_out = dram.tile([D, 512], BF, tag="arout")
            nc.gpsimd.collective_compute(
                "AllReduce", ALU.add, replica_groups=RG,
                ins=[ar_in.opt()], outs=[ar_out.opt()],
            )
            return ar_out

        def load_cols(param, l, tag):
            t = bpool.tile([128, KT], F32, tag=tag)
            nc.sync.dma_start(t, param[l].rearrange("(m p) -> p m", p=128))
            return t

        # ---- layer loop; FFN AllReduces carried into the next iteration ----
        ar_ffn = [None, None]
        gb_prev = None
        for l in range(N_LAYERS):
            wqkv_sb = wq_pool.tile([128, KT, 768], BF, tag="wqkv")
            nc.sync.dma_start(wqkv_sb, _r2(wqkv[l]))
            wo_sb = wq_pool.tile([128, 2, D], BF, tag="wo")
            nc.sync.dma_start(wo_sb, _r2(wo[l]))
            w1_sb = wf_pool.tile([128, KT, FFL], BF, tag="w1")
            nc.sync.dma_start(w1_sb, _r2(w1[l]))
            w2_sb = wf_pool.tile([128, KT, D], BF, tag="w2")
            nc.sync.dma_start(w2_sb, _r2(w2[l]))
            bqk_sb = bpool.tile([128, 4], F32, tag="bqk")
            nc.sync.dma_start(bqk_sb, bqk[l].rearrange("(m p) -> p m", p=128))
            ev1_sb = load_cols(evb1, l, "ev1")
            ev2_sb = load_cols(evb2, l, "ev2")
            ga_sb = load_cols(resga, l, "ga")     # gamma2[l-1] (ones at l=0)
            gb_sb = load_cols(resgb, l, "gb")     # gamma1[l]
            b1_sb = load_cols(b1p, l, "b1")

            # LN2 of previous layer (consumes prev FFN ARs), then this layer's QKV
            for nb in range(NB):
                if l > 0:
                    ln_block(nb, ar_ffn[nb], gb_prev)
                qkv_block(nb, wqkv_sb, bqk_sb)

            ar_attn = [None, None]
            for nb in range(NB):
                attn_block(nb)
                ar_attn[nb] = outproj_block(nb, wo_sb, ev1_sb)

            for nb in range(NB):
                ln_block(nb, ar_attn[nb], ga_sb)
                ar_ffn[nb] = ffn_block(nb, w1_sb, w2_sb, b1_sb, ev2_sb)
            gb_prev = gb_sb

        # ---- final LN2, vocab projection, per-core sum-exp ----
        bout_sb = const.tile([128, NVM], F32)
        nc.sync.dma_start(bout_sb, bout.rearrange("(m p) -> p m", p=128))
        for nb in range(NB):
            ln_block(nb, ar_ffn[nb], gb_prev)
            tsl = slice(nb * 512, (nb + 1) * 512)
            ps_acc = st_psum.tile([1, 512], F32, tag="st1")
            for vm in range(NVM):
                wv_sb = wch_pool.tile([128, KT, 128], BF, tag="wch")
                nc.sync.dma_start(wv_sb, _r2(wout)[:, :, vm * 128 : (vm + 1) * 128])
                ps = mm_psum.tile([128, 512], F32, tag="mm")
                for kt in range(KT):
                    nc.tensor.matmul(
                        ps, wv_sb[:, kt, :], hb[:, kt, tsl],
                        start=(kt == 0), stop=(kt == KT - 1),
                    )
                outf = spool.tile([128, 512], F32, tag="outf")
                evict(ps, outf, col=bout_sb[:, vm : vm + 1], eng=RR[vm % 2])
                nc.sync.dma_start(out[vm * 128 : (vm + 1) * 128, tsl], outf)
                eb = spool.tile([128, 512], BF, tag="eb")
                nc.scalar.activation(eb, ps, AF.Exp, bias=bout_sb[:, vm : vm + 1])
                nc.tensor.matmul(
                    ps_acc, ones_1, eb,
                    start=(vm == 0), stop=(vm == NVM - 1), skip_group_check=True,
                )
            se_row = rpool.tile([1, 512], F32, tag="serow")
            nc.vector.tensor_copy(se_row, ps_acc)
            nc.sync.dma_start(sumexp[nb : nb + 1, :], se_row)

    nsplit = split_sync_waits(nc)
    print(f"split_sync_waits: {nsplit} NOPs inserted")
    return nc


def _bf16(a):
    return np.asarray(a, dtype=ml_dtypes.bfloat16)


def make_in_maps(x, tok_emb, pos_emb, wq, bq, wk, bk, wv, bv, wo, bo,
                 ln1_g, ln1_b, w1, b1, w2, b2, ln2_g, ln2_b, w_out, b_out):
    """Shard full inputs -> per-core input maps (with host-side folds)."""
    LE = wq.shape[0]
    f32 = np.float32
    # gamma2/beta2 of the *previous* layer (identity for layer 0)
    ga = np.concatenate([np.ones((1, D), f32), ln2_g[:-1]], axis=0)   # [L, D]
    be = np.concatenate([np.zeros((1, D), f32), ln2_b[:-1]], axis=0)  # [L, D]
    per_r = []
    for r in range(TP):
        hs = slice(HL * r, HL * (r + 1))
        # per-head weights, head-major concat, gamma-prev scaled rows
        wq_r = wq[:, hs].transpose(0, 2, 1, 3).reshape(LE, D, HL * DH)
        wk_r = wk[:, hs].transpose(0, 2, 1, 3).reshape(LE, D, HL * DH)
        wv_r = wv[:, hs].transpose(0, 2, 1, 3).reshape(LE, D, HL * DH)
        wqkv_r = np.concatenate([wq_r, wk_r, wv_r], axis=2) * ga[:, :, None]
        # bias folds: b' = b + W^T beta_prev
        bq_r = bq[:, hs].reshape(LE, -1) + np.einsum('ldm,ld->lm', wq_r, be)
        bk_r = bk[:, hs].reshape(LE, -1) + np.einsum('ldm,ld->lm', wk_r, be)
        bv_r = bv[:, hs].reshape(LE, -1) + np.einsum('ldm,ld->lm', wv_r, be)
        bqk_r = np.concatenate([bq_r, bk_r], axis=1).astype(f32)      # [L, 512]
        wo_r = wo[:, DH * HL * r : DH * HL * (r + 1), :]              # [L, 256, D]
        # eviction biases (pre-AllReduce, so /TP; plus folded V-bias through wo)
        ev1_r = (bo + be) / TP + np.einsum('lcd,lc->ld', wo_r, bv_r)
        ev2_r = (b2 + ln1_b) / TP
        fs = slice(FFL * r, FFL * (r + 1))
        w1_r = w1[:, :, fs] * ln1_g[:, :, None]
        b1_r = b1[:, fs] + np.einsum('ldm,ld->lm', w1[:, :, fs], ln1_b)
        vs = slice(VL * r, VL * (r + 1))
        wout_r = np.zeros((D, VLP), f32)
        wout_r[:, :VL] = w_out[:, vs] * ln2_g[-1][:, None]
        bout_r = np.full((VLP,), -1e30, f32)
        bout_r[:VL] = b_out[vs] + w_out[:, vs].T @ ln2_b[-1]
        per_r.append(dict(
            wqkv=_bf16(wqkv_r),
            bqk=np.ascontiguousarray(bqk_r),
            wo=_bf16(wo_r),
            evb1=np.ascontiguousarray(ev1_r, f32),
            evb2=np.ascontiguousarray(ev2_r, f32),
            resga=np.ascontiguousarray(ga, f32),
            resgb=np.ascontiguousarray(ln1_g, f32),
            w1=_bf16(w1_r),
            b1p=np.ascontiguousarray(b1_r, f32),
            w2=_bf16(w2[:, fs, :]),
            wout=_bf16(wout_r),
            bout=bout_r,
        ))
    in_maps = []
    for c in range(8):
        g, r = c // TP, c % TP
        emb = tok_emb[x[g]] + pos_emb[:S]          # [S, D]
        m = dict(per_r[r])
        m["h0T"] = _bf16(np.ascontiguousarray(emb.T))
        in_maps.append(m)
    return in_maps


_CACHED = {}


def _install_ntff_shim():
    """Provide antenv.axon_hooks.get_axon_ntff_profile_hook via ctypes on
    libaxon_pjrt.so (this container's trn_rl_repo snapshot lacks the module)."""
    import sys
    import types
    import ctypes
    import contextlib

    if "antenv.axon_hooks" in sys.modules:
        return
    try:
        lib = ctypes.CDLL("/opt/axon/libaxon_pjrt.so")
    except OSError:
        return
    lib.axon_start_nrt_profile.restype = ctypes.c_int64
    lib.axon_start_nrt_profile.argtypes = [ctypes.c_char_p, ctypes.c_size_t]
    lib.axon_stop_nrt_profile.restype = ctypes.c_int64
    lib.axon_stop_nrt_profile.argtypes = [ctypes.c_char_p]

    def get_axon_ntff_profile_hook():
        @contextlib.contextmanager
        def hook(neff_dir, trace_model_indices):
            d = str(neff_dir).encode()
            if lib.axon_start_nrt_profile(d, len(d)) != 0:
                yield
                return
            try:
                yield
            finally:
                lib.axon_stop_nrt_profile(d)

        return hook

    mod = types.ModuleType("antenv.axon_hooks")
    mod.get_axon_ntff_profile_hook = get_axon_ntff_profile_hook
    sys.modules["antenv.axon_hooks"] = mod


def kernel(**inputs):
    inputs = {k: np.asarray(v) for k, v in inputs.items()}
    if "nc" not in _CACHED:
        _CACHED["nc"] = build_program()
    nc = _CACHED["nc"]
    in_maps = make_in_maps(**inputs)
    trace = os.environ.get("BASS_GPT_TRACE", "0") == "1"
    if trace:
        _install_ntff_shim()
    res = run_bass_kernel_spmd(
        nc, in_maps, core_ids=list(range(8)), trace=trace,
    )
    if trace:
        print(f"HW exec time: {res.exec_time_ns} ns")
        _CACHED["last_result"] = res
    results = res.results
    full = np.empty((B, S, V), np.float32)
    for g in range(B):
        se = np.zeros((T,), np.float64)
        for r in range(TP):
            se += results[g * TP + r]["sumexp"].reshape(T).astype(np.float64)
        lse = np.log(se).astype(np.float32)                  # [T]
        for r in range(TP):
            c = g * TP + r
            full[g, :, VL * r : VL * (r + 1)] = (
                results[c]["out"][:VL, :] - lse[None, :]
            ).T
    return full


# revision 26
# speedup vs baseline: 1.5447x; 1.0813x over previous
"""GPT-style transformer forward on 8 Trainium2 NeuronCores.

Sharding: data-parallel over batch (2 groups of 4 cores), tensor-parallel
within each group (heads / FFN hidden / vocab columns split 4 ways).
Device activations are feature-major [feature, token] so all matmuls run
without transposes.

Schedule: per layer the token dim is split into two 512-blocks; each
block's AllReduce is issued while the other block computes (attention,
FFN, or the next layer's QKV), hiding collective latency.  LN gamma/beta
and all linear biases are folded into weights / eviction biases host-side,
so the device residual+LN is: fused residual-scale-add (gpsimd STT) ->
sum/sumsq ones-matmuls -> rsqrt -> two broadcast-apply ops.  Causal
attention computes only unmasked column ranges; diagonal tiles are masked
in-place with affine_select.  The log-softmax denominator is reduced on
device (per-core sum-exp) and the final subtract is folded into the
host-side unshard.
"""

import os
from contextlib import ExitStack

import numpy as np
import ml_dtypes

import concourse.bass as bass
import concourse.mybir as mybir
import concourse.tile as tile
from concourse.bass_utils import run_bass_kernel_spmd
from concourse.vector_clock import ScopedClock


def _drain_and_barrier(self, tick_clock, wait_clock):
    """The walrus build here encodes Drain/NoOp as TPB_CTRL with at most one
    sync-wait slot; Tile's stock tail attaches all outstanding waits to the
    Drain and fails codegen. Split the waits one-per-NOP instead."""
    nop_inst = self.nc.sync.nop(nofuse=True)
    wait_clock.add_sem_waits(nop_inst.ins, ScopedClock({None: tick_clock.global_clock}))
    si = nop_inst.ins.sync_info
    if si is not None and len(si.on_wait) > 1:
        waits = list(si.on_wait)
        nop_inst.ins.sync_info = mybir.SyncInfo(on_wait=waits[:1], on_update=list(si.on_update))
        for w in waits[1:]:
            n2 = self.nc.sync.nop(nofuse=True)
            n2.ins.sync_info = mybir.SyncInfo(on_wait=[w], on_update=[])
    self.nc.sync.drain()
    self.nc.all_engine_barrier()
    assert self.sems is not None
    popped = self.nc._tile_sem_poison_stack.pop()
    assert popped is self._sem_poison
    self.nc.clear_and_free_semaphores(list(self.sems.allocated().values()))
    self.nc.all_engine_barrier()


tile.TileContext._drain_and_barrier = _drain_and_barrier

_MAX_WAITS = 1  # this walrus build caps sync-waits per instruction


def split_sync_waits(nc):
    """Hoist excess on_wait entries onto same-engine NOPs inserted before the
    instruction (engine queues execute in program order, so semantics hold)."""
    n = 0
    for bb in nc.main_func.blocks:
        insts = bb.instructions
        new_list = []
        for inst in insts:
            si = getattr(inst, "sync_info", None)
            if si is not None and len(si.on_wait) > _MAX_WAITS:
                waits = list(si.on_wait)
                for w in waits[:-_MAX_WAITS]:
                    n += 1
                    new_list.append(mybir.InstNoOp(
                        name=f"{inst.name}-sw{n}",
                        sync_info=mybir.SyncInfo(on_wait=[w], on_update=[]),
                        bass_nofuse=True,
                        engine=inst.engine,
                    ))
                inst.sync_info = mybir.SyncInfo(
                    on_wait=waits[-_MAX_WAITS:], on_update=list(si.on_update)
                )
            new_list.append(inst)
        if len(new_list) != len(insts):
            bb.instructions[:] = new_list
    return n


# Model dims (hardcoded per problem spec)
L_FULL, H, D, V, SMAX = 8, 16, 1024, 32000, 1024
DH = D // H          # 64
FF = 4 * D           # 4096
B, S = 2, 1024
T = S                # tokens per group (one batch element per group)
TP = 4               # tensor-parallel degree within a group
HL = H // TP         # 4 local heads
FFL = FF // TP       # 1024 local FFN cols
VL = V // TP         # 8000 local vocab cols
VLP = 8064           # padded to 63*128
NVM = VLP // 128     # 63 vocab m-tiles
EPS = 1e-5
KT = D // 128        # 8 k-tiles over model dim
NB = T // 512        # 2 token blocks of 512

BF = mybir.dt.bfloat16
F32 = mybir.dt.float32
AF = mybir.ActivationFunctionType
ALU = mybir.AluOpType

RG = [[0, 1, 2, 3], [4, 5, 6, 7]]

N_LAYERS = int(os.environ.get("BASS_GPT_LAYERS", str(L_FULL)))
SKIP_FINAL = os.environ.get("BASS_GPT_SKIP_FINAL", "0") == "1"


def _r2(ap):
    """[ (kt p) n ] -> [p kt n] view of a DRAM 2-D tensor (p=128)."""
    return ap.rearrange("(kt p) n -> p kt n", p=128)


def build_program():
    nc = bass.Bass("TRN2")

    # ---- DRAM parameters (per-core shards; gamma/beta/bias folded host-side) ----
    h0T = nc.declare_dram_parameter("h0T", [D, T], BF, isOutput=False)
    wqkv = nc.declare_dram_parameter("wqkv", [N_LAYERS, D, 3 * HL * DH], BF, isOutput=False)
    bqk = nc.declare_dram_parameter("bqk", [N_LAYERS, 4 * 128], F32, isOutput=False)
    wo = nc.declare_dram_parameter("wo", [N_LAYERS, HL * DH, D], BF, isOutput=False)
    evb1 = nc.declare_dram_parameter("evb1", [N_LAYERS, D], F32, isOutput=False)
    evb2 = nc.declare_dram_parameter("evb2", [N_LAYERS, D], F32, isOutput=False)
    resga = nc.declare_dram_parameter("resga", [N_LAYERS, D], F32, isOutput=False)
    resgb = nc.declare_dram_parameter("resgb", [N_LAYERS, D], F32, isOutput=False)
    w1 = nc.declare_dram_parameter("w1", [N_LAYERS, D, FFL], BF, isOutput=False)
    b1p = nc.declare_dram_parameter("b1p", [N_LAYERS, FFL], F32, isOutput=False)
    w2 = nc.declare_dram_parameter("w2", [N_LAYERS, FFL, D], BF, isOutput=False)
    wout = nc.declare_dram_parameter("wout", [D, VLP], BF, isOutput=False)
    bout = nc.declare_dram_parameter("bout", [VLP], F32, isOutput=False)
    out = nc.declare_dram_parameter("out", [VLP, T], F32, isOutput=True)
    sumexp = nc.declare_dram_parameter("sumexp", [NB, 512], F32, isOutput=True)

    with ExitStack() as ctx:
        tc = ctx.enter_context(tile.TileContext(nc))

        const = ctx.enter_context(tc.tile_pool(name="const", bufs=1))
        hpool = ctx.enter_context(tc.tile_pool(name="hpool", bufs=1))
        apool = ctx.enter_context(tc.tile_pool(name="apool", bufs=1))
        epool = ctx.enter_context(tc.tile_pool(name="epool", bufs=2))
        s1pool = ctx.enter_context(tc.tile_pool(name="s1pool", bufs=1))
        wq_pool = ctx.enter_context(tc.tile_pool(name="wq_pool", bufs=2))
        wf_pool = ctx.enter_context(tc.tile_pool(name="wf_pool", bufs=1))
        wch_pool = ctx.enter_context(tc.tile_pool(name="wch_pool", bufs=3))
        bpool = ctx.enter_context(tc.tile_pool(name="bpool", bufs=2))
        spool = ctx.enter_context(tc.tile_pool(name="spool", bufs=2))
        rpool = ctx.enter_context(tc.tile_pool(name="rpool", bufs=2))
        fpool = ctx.enter_context(tc.tile_pool(name="fpool", bufs=1))

        mm_psum = ctx.enter_context(tc.tile_pool(name="mm_psum", bufs=3, space="PSUM"))
        o_psum = ctx.enter_context(tc.tile_pool(name="o_psum", bufs=2, space="PSUM"))
        st_psum = ctx.enter_context(tc.tile_pool(name="st_psum", bufs=2, space="PSUM"))
        bc_psum = ctx.enter_context(tc.tile_pool(name="bc_psum", bufs=1, space="PSUM"))

        dram = ctx.enter_context(tc.tile_pool(name="dram", bufs=3, space="DRAM"))

        # ---- constants ----
        ones_d = const.tile([128, 1], BF)       # partition-sum lhsT, scaled 1/D (LN stats)
        nc.vector.memset(ones_d, 1.0 / D)
        ones_1 = const.tile([128, 1], BF)       # partition-sum lhsT (softmax denominator)
        nc.vector.memset(ones_1, 1.0)
        ones_m = const.tile([1, 128], BF)       # broadcast lhsT (K=1, M=128)
        nc.vector.memset(ones_m, 1.0)
        negones_m = const.tile([1, 128], BF)    # negated broadcast lhsT
        nc.vector.memset(negones_m, -1.0)
        eps_sb = const.tile([1, 1], F32)
        nc.vector.memset(eps_sb, float(EPS))

        # ---- persistent activation state ----
        hb = hpool.tile([128, KT, T], BF)       # residual stream (feature-major, pre-gamma/beta)
        qk_sb = apool.tile([128, 2, 2, T], BF)  # [part, q/k, head-pair, t]
        vaug = apool.tile([128, KT, HL, 65], BF)  # token-major V + ones col
        oT = apool.tile([128, 2, T], BF)        # attn head outputs (feature-major, normalized)
        f1 = fpool.tile([128, KT, T], BF)       # FFN hidden (local)

        nc.sync.dma_start(hb, _r2(h0T))         # h0 straight into the residual stream
        nc.vector.memset(vaug[:, :, :, 64:65], 1.0)

        def ln_block(nb, ar_out, gcol):
            """hb[:, :, tsl] <- normalize(hb * gcol + AR result) (token block nb)."""
            tsl = slice(nb * 512, (nb + 1) * 512)
            arb = s1pool.tile([128, KT, 512], BF, tag="arb")
            nc.sync.dma_start(arb, _r2(ar_out))
            xb = s1pool.tile([128, KT, 512], BF, tag="xb")
            ps_st = st_psum.tile([65, 512], F32, tag="st")
            for kt in range(KT):
                nc.vector.scalar_tensor_tensor(
                    out=xb[:, kt, :], in0=hb[:, kt, tsl],
                    scalar=gcol[:, kt : kt + 1], in1=arb[:, kt, :],
                    op0=ALU.mult, op1=ALU.add,
                )
                xsq = spool.tile([128, 512], BF, tag="xsq")
                nc.scalar.activation(xsq, xb[:, kt, :], AF.Square)
                nc.tensor.matmul(ps_st[0:1, :], ones_d, xb[:, kt, :],
                                 start=(kt == 0), stop=(kt == KT - 1),
                                 skip_group_check=True)
                nc.tensor.matmul(ps_st[64:65, :], ones_d, xsq,
                                 start=(kt == 0), stop=(kt == KT - 1),
                                 skip_group_check=True)
            # u = m2 - mu^2 ; a = rsqrt(u + eps) = exp(-0.5 ln(u + eps))
            negs1r = rpool.tile([1, 512], BF, tag="negs1r")
            nc.vector.tensor_scalar(out=negs1r, in0=ps_st[0:1, :], scalar1=-1.0,
                                    scalar2=None, op0=ALU.mult)
            t0n = rpool.tile([1, 512], F32, tag="t0n")
            nc.vector.tensor_mul(t0n, negs1r, ps_st[0:1, :])       # -mu^2
            u = rpool.tile([1, 512], F32, tag="u")
            nc.vector.tensor_add(u, t0n, ps_st[64:65, :])
            lnr = rpool.tile([1, 512], F32, tag="lnr")
            nc.scalar.activation(lnr, u, AF.Ln, bias=eps_sb[0:1, 0:1])
            a_row = rpool.tile([1, 512], BF, tag="arow")
            nc.scalar.activation(a_row, lnr, AF.Exp, scale=-0.5)
            # broadcast -mu and a across partitions via K=1 matmuls, stage in SBUF
            ps_nm = bc_psum.tile([128, 512], F32, tag="bc")
            nc.tensor.matmul(ps_nm, ones_m, negs1r, start=True, stop=True)
            nmb = spool.tile([128, 512], BF, tag="nmb")
            nc.scalar.activation(nmb, ps_nm, AF.Identity)
            ps_ab = bc_psum.tile([128, 512], F32, tag="bc")
            nc.tensor.matmul(ps_ab, ones_m, a_row, start=True, stop=True)
            ab = spool.tile([128, 512], BF, tag="ab")
            nc.scalar.activation(ab, ps_ab, AF.Identity)
            for kt in range(KT):
                tt = spool.tile([128, 512], BF, tag="lnt")
                e_add = nc.vector if kt % 2 == 0 else nc.gpsimd
                e_mul = nc.gpsimd if kt % 2 == 0 else nc.vector
                e_add.tensor_add(tt, xb[:, kt, :], nmb)
                e_mul.tensor_mul(hb[:, kt, tsl], tt, ab)

        def evict(ps, out_ap, col=None, relu=False, eng="scalar"):
            """PSUM -> SBUF eviction with optional per-partition bias / relu.
            (gpsimd cannot read PSUM, so only scalar/vector qualify.)"""
            if eng == "scalar":
                nc.scalar.activation(out_ap, ps, AF.Relu if relu else AF.Identity,
                                     bias=col if col is not None else 0.0)
            else:
                e = nc.vector
                if relu:
                    e.tensor_scalar(out=out_ap, in0=ps,
                                    scalar1=col if col is not None else 0.0,
                                    scalar2=0.0, op0=ALU.add, op1=ALU.max)
                elif col is not None:
                    e.tensor_scalar(out=out_ap, in0=ps, scalar1=col, scalar2=None,
                                    op0=ALU.add)
                else:
                    e.tensor_copy(out_ap, ps)

        RR = ("scalar", "vector")

        def qkv_block(nb, wqkv_sb, bqk_sb):
            tsl = slice(nb * 512, (nb + 1) * 512)
            for io in range(2):        # 0=q, 1=k  (feature-major out)
                for mt in range(2):    # head pair
                    mcol = (io * 2 + mt) * 128
                    ps = mm_psum.tile([128, 512], F32, tag="mm")
                    for kt in range(KT):
                        nc.tensor.matmul(
                            ps, wqkv_sb[:, kt, mcol : mcol + 128], hb[:, kt, tsl],
                            start=(kt == 0), stop=(kt == KT - 1),
                        )
                    evict(ps, qk_sb[:, io, mt, tsl],
                          col=bqk_sb[:, io * 2 + mt : io * 2 + mt + 1],
                          eng=RR[(io * 2 + mt) % 2])
            for tm in range(nb * 4, nb * 4 + 4):   # v, token-major
                ps = mm_psum.tile([128, 512], F32, tag="mm")
                for kt in range(KT):
                    nc.tensor.matmul(
                        ps[:, 0:256], hb[:, kt, tm * 128 : (tm + 1) * 128],
                        wqkv_sb[:, kt, 512:768],
                        start=(kt == 0), stop=(kt == KT - 1),
                    )
                evict(ps[:, 0:256].rearrange("p (h e) -> p h e", h=HL),
                      vaug[:, tm, :, 0:64], eng=RR[tm % 2])

        def attn_block(blk):
            t1base = blk * 512
            t2max = 4 * (blk + 1)
            for h in range(HL):
                prow = slice(64 * (h % 2), 64 * (h % 2) + 64)
                hm = h // 2
                et = epool.tile([128, 8, 512], BF, tag="eT")
                for t2t in range(t2max):
                    a = max(0, 128 * (t2t - 4 * blk))
                    ps = mm_psum.tile([128, 512], F32, tag="mm")
                    nc.tensor.matmul(
                        ps[:, a:],
                        qk_sb[prow, 1, hm, t2t * 128 : (t2t + 1) * 128],
                        qk_sb[prow, 0, hm, t1base + a : t1base + 512],
                        start=True, stop=True,
                    )
                    nc.scalar.activation(et[:, t2t, a:], ps[:, a:], AF.Exp, scale=0.125)
                    j = t2t - 4 * blk
                    if 0 <= j <= 3:
                        # diagonal 128x128 triangle: keep where t1 - t2 >= 0
                        nc.gpsimd.affine_select(
                            out=et[:, t2t, a : a + 128], in_=et[:, t2t, a : a + 128],
                            compare_op=ALU.is_ge, fill=0.0,
                            base=0, pattern=[[1, 128]], channel_multiplier=-1,
                        )
                ps_o = o_psum.tile([65, 512], F32, tag="o")
                for t2t in range(t2max):
                    a = max(0, 128 * (t2t - 4 * blk))
                    nc.tensor.matmul(
                        ps_o[:, a:], vaug[:, t2t, h, :], et[:, t2t, a:],
                        start=(t2t == 0), stop=(t2t == t2max - 1),
                        skip_group_check=True,
                    )
                # 1/den = exp(-ln(den)); broadcast over the 64 head dims via PE
                lnd = rpool.tile([1, 512], F32, tag="lnd")
                nc.scalar.activation(lnd, ps_o[64:65, :], AF.Ln)
                rec = rpool.tile([1, 512], BF, tag="rec")
                nc.scalar.activation(rec, lnd, AF.Exp, scale=-1.0)
                ps_rb = bc_psum.tile([128, 512], F32, tag="bc")
                nc.tensor.matmul(ps_rb[0:64, :], ones_m[:, 0:64], rec,
                                 start=True, stop=True)
                osb = spool.tile([64, 512], BF, tag="osb")
                nc.vector.tensor_copy(osb, ps_o[0:64, :])
                nc.vector.tensor_mul(oT[prow, hm, t1base : t1base + 512],
                                     osb, ps_rb[0:64, :])

        def outproj_block(nb, wo_sb, ev1_sb):
            tsl = slice(nb * 512, (nb + 1) * 512)
            ar_in = dram.tile([D, 512], BF, tag="arin")
            for mt in range(KT):
                ps = mm_psum.tile([128, 512], F32, tag="mm")
                for kt in range(2):
                    nc.tensor.matmul(
                        ps, wo_sb[:, kt, mt * 128 : (mt + 1) * 128], oT[:, kt, tsl],
                        start=(kt == 0), stop=(kt == 1),
                    )
                ob = spool.tile([128, 512], BF, tag="ob")
                evict(ps, ob, col=ev1_sb[:, mt : mt + 1], eng=RR[mt % 2])
                nc.sync.dma_start(ar_in[mt * 128 : (mt + 1) * 128, :], ob)
            ar_out = dram.tile([D, 512], BF, tag="arout")
            nc.gpsimd.collective_compute(
                "AllReduce", ALU.add, replica_groups=RG,
                ins=[ar_in.opt()], outs=[ar_out.opt()],
            )
            return ar_out

        def ffn_block(nb, w1_sb, w2_sb, b1_sb, ev2_sb):
            tsl = slice(nb * 512, (nb + 1) * 512)
            for mt in range(KT):
                ps = mm_psum.tile([128, 512], F32, tag="mm")
                for kt in range(KT):
                    nc.tensor.matmul(
                        ps, w1_sb[:, kt, mt * 128 : (mt + 1) * 128], hb[:, kt, tsl],
                        start=(kt == 0), stop=(kt == KT - 1),
                    )
                evict(ps, f1[:, mt, tsl], col=b1_sb[:, mt : mt + 1], relu=True,
                      eng=RR[mt % 2])
            ar_in = dram.tile([D, 512], BF, tag="arin")
            for mt in range(KT):
                ps = mm_psum.tile([128, 512], F32, tag="mm")
                for kt in range(KT):
                    nc.tensor.matmul(
                        ps, w2_sb[:, kt, mt * 128 : (mt + 1) * 128], f1[:, kt, tsl],
                        start=(kt == 0), stop=(kt == KT - 1),
                    )
                ob = spool.tile([128, 512], BF, tag="ob")
                evict(ps, ob, col=ev2_sb[:, mt : mt + 1], eng=RR[(mt + 1) % 2])
                nc.sync.dma_start(ar_in[mt * 128 : (mt + 1) * 128, :], ob)
            ar_out = dram.tile([D, 512], BF, tag="arout")
            nc.gpsimd.collective_compute(
                "AllReduce", ALU.add, replica_groups=RG,
                ins=[ar_in.opt()], outs=[ar_out.opt()],
            )
            return ar_out

        def load_cols(param, l, tag):
            t = bpool.tile([128, KT], F32, tag=tag)
            nc.sync.dma_start(t, param[l].rearrange("(m p) -> p m", p=128))
            return t

        # ---- layer loop; FFN AllReduces carried into the next iteration ----
        ar_ffn = [None, None]
        gb_prev = None
        for l in range(N_LAYERS):
            wqkv_sb = wq_pool.tile([128, KT, 768], BF, tag="wqkv")
            nc.sync.dma_start(wqkv_sb, _r2(wqkv[l]))
            wo_sb = wq_pool.tile([128, 2, D], BF, tag="wo")
            nc.sync.dma_start(wo_sb, _r2(wo[l]))
            w1_sb = wf_pool.tile([128, KT, FFL], BF, tag="w1")
            nc.sync.dma_start(w1_sb, _r2(w1[l]))
            w2_sb = wf_pool.tile([128, KT, D], BF, tag="w2")
            nc.sync.dma_start(w2_sb, _r2(w2[l]))
            bqk_sb = bpool.tile([128, 4], F32, tag="bqk")
            nc.sync.dma_start(bqk_sb, bqk[l].rearrange("(m p) -> p m", p=128))
            ev1_sb = load_cols(evb1, l, "ev1")
            ev2_sb = load_cols(evb2, l, "ev2")
            ga_sb = load_cols(resga, l, "ga")     # gamma2[l-1] (ones at l=0)
            gb_sb = load_cols(resgb, l, "gb")     # gamma1[l]
            b1_sb = load_cols(b1p, l, "b1")

            # LN2 of previous layer (consumes prev FFN ARs), then this layer's QKV
            for nb in range(NB):
                if l > 0:
                    ln_block(nb, ar_ffn[nb], gb_prev)
                qkv_block(nb, wqkv_sb, bqk_sb)

            ar_attn = [None, None]
            for nb in range(NB):
                attn_block(nb)
                ar_attn[nb] = outproj_block(nb, wo_sb, ev1_sb)

            for nb in range(NB):
                ln_block(nb, ar_attn[nb], ga_sb)
                ar_ffn[nb] = ffn_block(nb, w1_sb, w2_sb, b1_sb, ev2_sb)
            gb_prev = gb_sb

        # ---- final LN2, vocab projection, per-core sum-exp ----
        bout_sb = const.tile([128, NVM], F32)
        nc.sync.dma_start(bout_sb, bout.rearrange("(m p) -> p m", p=128))
        for nb in range(NB):
            ln_block(nb, ar_ffn[nb], gb_prev)
            tsl = slice(nb * 512, (nb + 1) * 512)
            ps_acc_t = st_psum.tile([65, 512], F32, tag="st")
            ps_acc = ps_acc_t[0:1, :]
            for vm in range(NVM):
                wv_sb = wch_pool.tile([128, KT, 128], BF, tag="wch")
                nc.sync.dma_start(wv_sb, _r2(wout)[:, :, vm * 128 : (vm + 1) * 128])
                ps = mm_psum.tile([128, 512], F32, tag="mm")
                for kt in range(KT):
                    nc.tensor.matmul(
                        ps, wv_sb[:, kt, :], hb[:, kt, tsl],
                        start=(kt == 0), stop=(kt == KT - 1),
                    )
                outf = spool.tile([128, 512], F32, tag="outf")
                evict(ps, outf, col=bout_sb[:, vm : vm + 1], eng=RR[vm % 2])
                nc.sync.dma_start(out[vm * 128 : (vm + 1) * 128, tsl], outf)
                eb = spool.tile([128, 512], BF, tag="eb")
                nc.scalar.activation(eb, ps, AF.Exp, bias=bout_sb[:, vm : vm + 1])
                nc.tensor.matmul(
                    ps_acc, ones_1, eb,
                    start=(vm == 0), stop=(vm == NVM - 1), skip_group_check=True,
                )
            se_row = rpool.tile([1, 512], F32, tag="serow")
            nc.vector.tensor_copy(se_row, ps_acc)
            nc.sync.dma_start(sumexp[nb : nb + 1, :], se_row)

    nsplit = split_sync_waits(nc)
    print(f"split_sync_waits: {nsplit} NOPs inserted")
    return nc


def _bf16(a):
    return np.asarray(a, dtype=ml_dtypes.bfloat16)


def make_in_maps(x, tok_emb, pos_emb, wq, bq, wk, bk, wv, bv, wo, bo,
                 ln1_g, ln1_b, w1, b1, w2, b2, ln2_g, ln2_b, w_out, b_out):
    """Shard full inputs -> per-core input maps (with host-side folds)."""
    LE = wq.shape[0]
    f32 = np.float32
    # gamma2/beta2 of the *previous* layer (identity for layer 0)
    ga = np.concatenate([np.ones((1, D), f32), ln2_g[:-1]], axis=0)   # [L, D]
    be = np.concatenate([np.zeros((1, D), f32), ln2_b[:-1]], axis=0)  # [L, D]
    per_r = []
    for r in range(TP):
        hs = slice(HL * r, HL * (r + 1))
        # per-head weights, head-major concat, gamma-prev scaled rows
        wq_r = wq[:, hs].transpose(0, 2, 1, 3).reshape(LE, D, HL * DH)
        wk_r = wk[:, hs].transpose(0, 2, 1, 3).reshape(LE, D, HL * DH)
        wv_r = wv[:, hs].transpose(0, 2, 1, 3).reshape(LE, D, HL * DH)
        wqkv_r = np.concatenate([wq_r, wk_r, wv_r], axis=2) * ga[:, :, None]
        # bias folds: b' = b + W^T beta_prev
        bq_r = bq[:, hs].reshape(LE, -1) + np.einsum('ldm,ld->lm', wq_r, be)
        bk_r = bk[:, hs].reshape(LE, -1) + np.einsum('ldm,ld->lm', wk_r, be)
        bv_r = bv[:, hs].reshape(LE, -1) + np.einsum('ldm,ld->lm', wv_r, be)
        bqk_r = np.concatenate([bq_r, bk_r], axis=1).astype(f32)      # [L, 512]
        wo_r = wo[:, DH * HL * r : DH * HL * (r + 1), :]              # [L, 256, D]
        # eviction biases (pre-AllReduce, so /TP; plus folded V-bias through wo)
        ev1_r = (bo + be) / TP + np.einsum('lcd,lc->ld', wo_r, bv_r)
        ev2_r = (b2 + ln1_b) / TP
        fs = slice(FFL * r, FFL * (r + 1))
        w1_r = w1[:, :, fs] * ln1_g[:, :, None]
        b1_r = b1[:, fs] + np.einsum('ldm,ld->lm', w1[:, :, fs], ln1_b)
        vs = slice(VL * r, VL * (r + 1))
        wout_r = np.zeros((D, VLP), f32)
        wout_r[:, :VL] = w_out[:, vs] * ln2_g[-1][:, None]
        bout_r = np.full((VLP,), -1e30, f32)
        bout_r[:VL] = b_out[vs] + w_out[:, vs].T @ ln2_b[-1]
        per_r.append(dict(
            wqkv=_bf16(wqkv_r),
            bqk=np.ascontiguousarray(bqk_r),
            wo=_bf16(wo_r),
            evb1=np.ascontiguousarray(ev1_r, f32),
            evb2=np.ascontiguousarray(ev2_r, f32),
            resga=np.ascontiguousarray(ga, f32),
            resgb=np.ascontiguousarray(ln1_g, f32),
            w1=_bf16(w1_r),
            b1p=np.ascontiguousarray(b1_r, f32),
            w2=_bf16(w2[:, fs, :]),
            wout=_bf16(wout_r),
            bout=bout_r,
        ))
    in_maps = []
    for c in range(8):
        g, r = c // TP, c % TP
        emb = tok_emb[x[g]] + pos_emb[:S]          # [S, D]
        m = dict(per_r[r])
        m["h0T"] = _bf16(np.ascontiguousarray(emb.T))
        in_maps.append(m)
    return in_maps


_CACHED = {}


def _install_ntff_shim():
    """Provide antenv.axon_hooks.get_axon_ntff_profile_hook via ctypes on
    libaxon_pjrt.so (this container's trn_rl_repo snapshot lacks the module)."""
    import sys
    import types
    import ctypes
    import contextlib

    if "antenv.axon_hooks" in sys.modules:
        return
    try:
        lib = ctypes.CDLL("/opt/axon/libaxon_pjrt.so")
    except OSError:
        return
    lib.axon_start_nrt_profile.restype = ctypes.c_int64
    lib.axon_start_nrt_profile.argtypes = [ctypes.c_char_p, ctypes.c_size_t]
    lib.axon_stop_nrt_profile.restype = ctypes.c_int64
    lib.axon_stop_nrt_profile.argtypes = [ctypes.c_char_p]

    def get_axon_ntff_profile_hook():
        @contextlib.contextmanager
        def hook(neff_dir, trace_model_indices):
            d = str(neff_dir).encode()
            if lib.axon_start_nrt_profile(d, len(d)) != 0:
                yield
                return
            try:
                yield
            finally:
                lib.axon_stop_nrt_profile(d)

        return hook

    mod = types.ModuleType("antenv.axon_hooks")
    mod.get_axon_ntff_profile_hook = get_axon_ntff_profile_hook
    sys.modules["antenv.axon_hooks"] = mod


def kernel(**inputs):
    inputs = {k: np.asarray(v) for k, v in inputs.items()}
    if "nc" not in _CACHED:
        _CACHED["nc"] = build_program()
    nc = _CACHED["nc"]
    in_maps = make_in_maps(**inputs)
    trace = os.environ.get("BASS_GPT_TRACE", "0") == "1"
    if trace:
        _install_ntff_shim()
    res = run_bass_kernel_spmd(
        nc, in_maps, core_ids=list(range(8)), trace=trace,
    )
    if trace:
        print(f"HW exec time: {res.exec_time_ns} ns")
        _CACHED["last_result"] = res
    results = res.results
    full = np.empty((B, S, V), np.float32)
    for g in range(B):
        se = np.zeros((T,), np.float64)
        for r in range(TP):
            se += results[g * TP + r]["sumexp"].reshape(T).astype(np.float64)
        lse = np.log(se).astype(np.float32)                  # [T]
        for r in range(TP):
            c = g * TP + r
            full[g, :, VL * r : VL * (r + 1)] = (
                results[c]["out"][:VL, :] - lse[None, :]
            ).T
    return full


# revision 27
# speedup vs baseline: 1.6287x; 1.0544x over previous
"""GPT-style transformer forward on 8 Trainium2 NeuronCores.

Sharding: data-parallel over batch (2 groups of 4 cores), tensor-parallel
within each group (heads / FFN hidden / vocab columns split 4 ways).
Device activations are feature-major [feature, token] so all matmuls run
without transposes.

Schedule: per layer the token dim is split into two 512-blocks; each
block's AllReduce is issued while the other block computes (attention,
FFN, or the next layer's QKV), hiding collective latency.  LN gamma/beta
and all linear biases are folded into weights / eviction biases host-side,
so the device residual+LN is: fused residual-scale-add (gpsimd STT) ->
sum/sumsq ones-matmuls -> rsqrt -> two broadcast-apply ops.  Causal
attention computes only unmasked column ranges; diagonal tiles are masked
in-place with affine_select.  The log-softmax denominator is reduced on
device (per-core sum-exp) and the final subtract is folded into the
host-side unshard.
"""

import os
from contextlib import ExitStack

import numpy as np
import ml_dtypes

import concourse.bass as bass
import concourse.mybir as mybir
import concourse.tile as tile
from concourse.bass_utils import run_bass_kernel_spmd
from concourse.vector_clock import ScopedClock


def _drain_and_barrier(self, tick_clock, wait_clock):
    """The walrus build here encodes Drain/NoOp as TPB_CTRL with at most one
    sync-wait slot; Tile's stock tail attaches all outstanding waits to the
    Drain and fails codegen. Split the waits one-per-NOP instead."""
    nop_inst = self.nc.sync.nop(nofuse=True)
    wait_clock.add_sem_waits(nop_inst.ins, ScopedClock({None: tick_clock.global_clock}))
    si = nop_inst.ins.sync_info
    if si is not None and len(si.on_wait) > 1:
        waits = list(si.on_wait)
        nop_inst.ins.sync_info = mybir.SyncInfo(on_wait=waits[:1], on_update=list(si.on_update))
        for w in waits[1:]:
            n2 = self.nc.sync.nop(nofuse=True)
            n2.ins.sync_info = mybir.SyncInfo(on_wait=[w], on_update=[])
    self.nc.sync.drain()
    self.nc.all_engine_barrier()
    assert self.sems is not None
    popped = self.nc._tile_sem_poison_stack.pop()
    assert popped is self._sem_poison
    self.nc.clear_and_free_semaphores(list(self.sems.allocated().values()))
    self.nc.all_engine_barrier()


tile.TileContext._drain_and_barrier = _drain_and_barrier

_MAX_WAITS = 1  # this walrus build caps sync-waits per instruction


def split_sync_waits(nc):
    """Hoist excess on_wait entries onto same-engine NOPs inserted before the
    instruction (engine queues execute in program order, so semantics hold)."""
    n = 0
    for bb in nc.main_func.blocks:
        insts = bb.instructions
        new_list = []
        for inst in insts:
            si = getattr(inst, "sync_info", None)
            if si is not None and len(si.on_wait) > _MAX_WAITS:
                waits = list(si.on_wait)
                for w in waits[:-_MAX_WAITS]:
                    n += 1
                    new_list.append(mybir.InstNoOp(
                        name=f"{inst.name}-sw{n}",
                        sync_info=mybir.SyncInfo(on_wait=[w], on_update=[]),
                        bass_nofuse=True,
                        engine=inst.engine,
                    ))
                inst.sync_info = mybir.SyncInfo(
                    on_wait=waits[-_MAX_WAITS:], on_update=list(si.on_update)
                )
            new_list.append(inst)
        if len(new_list) != len(insts):
            bb.instructions[:] = new_list
    return n


# Model dims (hardcoded per problem spec)
L_FULL, H, D, V, SMAX = 8, 16, 1024, 32000, 1024
DH = D // H          # 64
FF = 4 * D           # 4096
B, S = 2, 1024
T = S                # tokens per group (one batch element per group)
TP = 4               # tensor-parallel degree within a group
HL = H // TP         # 4 local heads
FFL = FF // TP       # 1024 local FFN cols
VL = V // TP         # 8000 local vocab cols
VLP = 8064           # padded to 63*128
NVM = VLP // 128     # 63 vocab m-tiles
EPS = 1e-5
KT = D // 128        # 8 k-tiles over model dim
NB = T // 512        # 2 token blocks of 512

BF = mybir.dt.bfloat16
F32 = mybir.dt.float32
AF = mybir.ActivationFunctionType
ALU = mybir.AluOpType

RG = [[0, 1, 2, 3], [4, 5, 6, 7]]

N_LAYERS = int(os.environ.get("BASS_GPT_LAYERS", str(L_FULL)))
SKIP_FINAL = os.environ.get("BASS_GPT_SKIP_FINAL", "0") == "1"


def _r2(ap):
    """[ (kt p) n ] -> [p kt n] view of a DRAM 2-D tensor (p=128)."""
    return ap.rearrange("(kt p) n -> p kt n", p=128)


def build_program():
    nc = bass.Bass("TRN2")

    # ---- DRAM parameters (per-core shards; gamma/beta/bias folded host-side) ----
    h0T = nc.declare_dram_parameter("h0T", [D, T], BF, isOutput=False)
    wqkv = nc.declare_dram_parameter("wqkv", [N_LAYERS, D, 3 * HL * DH], BF, isOutput=False)
    bqk = nc.declare_dram_parameter("bqk", [N_LAYERS, 4 * 128], F32, isOutput=False)
    wo = nc.declare_dram_parameter("wo", [N_LAYERS, HL * DH, D], BF, isOutput=False)
    evb1 = nc.declare_dram_parameter("evb1", [N_LAYERS, D], F32, isOutput=False)
    evb2 = nc.declare_dram_parameter("evb2", [N_LAYERS, D], F32, isOutput=False)
    resga = nc.declare_dram_parameter("resga", [N_LAYERS, D], F32, isOutput=False)
    resgb = nc.declare_dram_parameter("resgb", [N_LAYERS, D], F32, isOutput=False)
    w1 = nc.declare_dram_parameter("w1", [N_LAYERS, D, FFL], BF, isOutput=False)
    b1p = nc.declare_dram_parameter("b1p", [N_LAYERS, FFL], F32, isOutput=False)
    w2 = nc.declare_dram_parameter("w2", [N_LAYERS, FFL, D], BF, isOutput=False)
    wout = nc.declare_dram_parameter("wout", [D, VLP], BF, isOutput=False)
    bout = nc.declare_dram_parameter("bout", [VLP], F32, isOutput=False)
    out = nc.declare_dram_parameter("out", [VLP, T], F32, isOutput=True)
    sumexp = nc.declare_dram_parameter("sumexp", [NB, 512], F32, isOutput=True)

    with ExitStack() as ctx:
        tc = ctx.enter_context(tile.TileContext(nc))

        const = ctx.enter_context(tc.tile_pool(name="const", bufs=1))
        hpool = ctx.enter_context(tc.tile_pool(name="hpool", bufs=1))
        apool = ctx.enter_context(tc.tile_pool(name="apool", bufs=1))
        epool = ctx.enter_context(tc.tile_pool(name="epool", bufs=2))
        s1pool = ctx.enter_context(tc.tile_pool(name="s1pool", bufs=1))
        wq_pool = ctx.enter_context(tc.tile_pool(name="wq_pool", bufs=2))
        wf_pool = ctx.enter_context(tc.tile_pool(name="wf_pool", bufs=1))
        wch_pool = ctx.enter_context(tc.tile_pool(name="wch_pool", bufs=3))
        bpool = ctx.enter_context(tc.tile_pool(name="bpool", bufs=2))
        spool = ctx.enter_context(tc.tile_pool(name="spool", bufs=2))
        rpool = ctx.enter_context(tc.tile_pool(name="rpool", bufs=2))
        fpool = ctx.enter_context(tc.tile_pool(name="fpool", bufs=1))

        mm_psum = ctx.enter_context(tc.tile_pool(name="mm_psum", bufs=3, space="PSUM"))
        o_psum = ctx.enter_context(tc.tile_pool(name="o_psum", bufs=2, space="PSUM"))
        st_psum = ctx.enter_context(tc.tile_pool(name="st_psum", bufs=2, space="PSUM"))
        bc_psum = ctx.enter_context(tc.tile_pool(name="bc_psum", bufs=1, space="PSUM"))

        dram = ctx.enter_context(tc.tile_pool(name="dram", bufs=3, space="DRAM"))

        # ---- constants ----
        ones_d = const.tile([128, 1], BF)       # partition-sum lhsT, scaled 1/D (LN stats)
        nc.vector.memset(ones_d, 1.0 / D)
        ones_1 = const.tile([128, 1], BF)       # partition-sum lhsT (softmax denominator)
        nc.vector.memset(ones_1, 1.0)
        ones_m = const.tile([1, 128], BF)       # broadcast lhsT (K=1, M=128)
        nc.vector.memset(ones_m, 1.0)
        negones_m = const.tile([1, 128], BF)    # negated broadcast lhsT
        nc.vector.memset(negones_m, -1.0)
        eps_sb = const.tile([1, 1], F32)
        nc.vector.memset(eps_sb, float(EPS))

        # ---- persistent activation state ----
        hb = hpool.tile([128, KT, T], BF)       # residual stream (feature-major, pre-gamma/beta)
        qk_sb = apool.tile([128, 2, 2, T], BF)  # [part, q/k, head-pair, t]
        vaug = apool.tile([128, KT, HL, 65], BF)  # token-major V + ones col
        oT = apool.tile([128, 2, T], BF)        # attn head outputs (feature-major, normalized)
        f1 = fpool.tile([128, KT, T], BF)       # FFN hidden (local)

        nc.sync.dma_start(hb, _r2(h0T))         # h0 straight into the residual stream
        nc.vector.memset(vaug[:, :, :, 64:65], 1.0)

        def ln_block(nb, ar_out, gcol):
            """hb[:, :, tsl] <- normalize(hb * gcol + AR result) (token block nb)."""
            tsl = slice(nb * 512, (nb + 1) * 512)
            arb = s1pool.tile([128, KT, 512], BF, tag="arb")
            nc.sync.dma_start(arb, _r2(ar_out))
            xb = s1pool.tile([128, KT, 512], BF, tag="xb")
            ps_st = st_psum.tile([65, 512], F32, tag="st")
            for kt in range(KT):
                nc.vector.scalar_tensor_tensor(
                    out=xb[:, kt, :], in0=hb[:, kt, tsl],
                    scalar=gcol[:, kt : kt + 1], in1=arb[:, kt, :],
                    op0=ALU.mult, op1=ALU.add,
                )
                xsq = spool.tile([128, 512], BF, tag="xsq")
                nc.scalar.activation(xsq, xb[:, kt, :], AF.Square)
                nc.tensor.matmul(ps_st[0:1, :], ones_d, xb[:, kt, :],
                                 start=(kt == 0), stop=(kt == KT - 1),
                                 skip_group_check=True)
                nc.tensor.matmul(ps_st[64:65, :], ones_d, xsq,
                                 start=(kt == 0), stop=(kt == KT - 1),
                                 skip_group_check=True)
            # u = m2 - mu^2 ; a = rsqrt(u + eps) = exp(-0.5 ln(u + eps))
            negs1r = rpool.tile([1, 512], BF, tag="negs1r")
            nc.vector.tensor_scalar(out=negs1r, in0=ps_st[0:1, :], scalar1=-1.0,
                                    scalar2=None, op0=ALU.mult)
            t0n = rpool.tile([1, 512], F32, tag="t0n")
            nc.vector.tensor_mul(t0n, negs1r, ps_st[0:1, :])       # -mu^2
            u = rpool.tile([1, 512], F32, tag="u")
            nc.vector.tensor_add(u, t0n, ps_st[64:65, :])
            lnr = rpool.tile([1, 512], F32, tag="lnr")
            nc.scalar.activation(lnr, u, AF.Ln, bias=eps_sb[0:1, 0:1])
            a_row = rpool.tile([1, 512], BF, tag="arow")
            nc.scalar.activation(a_row, lnr, AF.Exp, scale=-0.5)
            # broadcast -mu and a across partitions via K=1 matmuls, stage in SBUF
            ps_nm = bc_psum.tile([128, 512], F32, tag="bc")
            nc.tensor.matmul(ps_nm, ones_m, negs1r, start=True, stop=True)
            nmb = spool.tile([128, 512], BF, tag="nmb")
            nc.scalar.activation(nmb, ps_nm, AF.Identity)
            ps_ab = bc_psum.tile([128, 512], F32, tag="bc")
            nc.tensor.matmul(ps_ab, ones_m, a_row, start=True, stop=True)
            ab = spool.tile([128, 512], BF, tag="ab")
            nc.scalar.activation(ab, ps_ab, AF.Identity)
            for kt in range(KT):
                tt = spool.tile([128, 512], BF, tag="lnt")
                e_add = nc.vector if kt % 2 == 0 else nc.gpsimd
                e_mul = nc.gpsimd if kt % 2 == 0 else nc.vector
                e_add.tensor_add(tt, xb[:, kt, :], nmb)
                e_mul.tensor_mul(hb[:, kt, tsl], tt, ab)

        def evict(ps, out_ap, col=None, relu=False, eng="scalar"):
            """PSUM -> SBUF eviction with optional per-partition bias / relu.
            (gpsimd cannot read PSUM, so only scalar/vector qualify.)"""
            if eng == "scalar":
                nc.scalar.activation(out_ap, ps, AF.Relu if relu else AF.Identity,
                                     bias=col if col is not None else 0.0)
            else:
                e = nc.vector
                if relu:
                    e.tensor_scalar(out=out_ap, in0=ps,
                                    scalar1=col if col is not None else 0.0,
                                    scalar2=0.0, op0=ALU.add, op1=ALU.max)
                elif col is not None:
                    e.tensor_scalar(out=out_ap, in0=ps, scalar1=col, scalar2=None,
                                    op0=ALU.add)
                else:
                    e.tensor_copy(out_ap, ps)

        RR = ("scalar", "vector")

        def qkv_block(nb, wqkv_sb, bqk_sb):
            tsl = slice(nb * 512, (nb + 1) * 512)
            for io in range(2):        # 0=q, 1=k  (feature-major out)
                for mt in range(2):    # head pair
                    mcol = (io * 2 + mt) * 128
                    ps = mm_psum.tile([128, 512], F32, tag="mm")
                    for kt in range(KT):
                        nc.tensor.matmul(
                            ps, wqkv_sb[:, kt, mcol : mcol + 128], hb[:, kt, tsl],
                            start=(kt == 0), stop=(kt == KT - 1),
                        )
                    evict(ps, qk_sb[:, io, mt, tsl],
                          col=bqk_sb[:, io * 2 + mt : io * 2 + mt + 1],
                          eng=RR[(io * 2 + mt) % 2])
            for tm in range(nb * 4, nb * 4 + 4):   # v, token-major
                ps = mm_psum.tile([128, 512], F32, tag="mm")
                for kt in range(KT):
                    nc.tensor.matmul(
                        ps[:, 0:256], hb[:, kt, tm * 128 : (tm + 1) * 128],
                        wqkv_sb[:, kt, 512:768],
                        start=(kt == 0), stop=(kt == KT - 1),
                    )
                evict(ps[:, 0:256].rearrange("p (h e) -> p h e", h=HL),
                      vaug[:, tm, :, 0:64], eng=RR[tm % 2])

        def attn_block(blk):
            t1base = blk * 512
            t2max = 4 * (blk + 1)
            for h in range(HL):
                prow = slice(64 * (h % 2), 64 * (h % 2) + 64)
                hm = h // 2
                et = epool.tile([128, 8, 512], BF, tag="eT")
                for t2t in range(t2max):
                    a = max(0, 128 * (t2t - 4 * blk))
                    ps = mm_psum.tile([128, 512], F32, tag="mm")
                    nc.tensor.matmul(
                        ps[:, a:],
                        qk_sb[prow, 1, hm, t2t * 128 : (t2t + 1) * 128],
                        qk_sb[prow, 0, hm, t1base + a : t1base + 512],
                        start=True, stop=True,
                    )
                    nc.scalar.activation(et[:, t2t, a:], ps[:, a:], AF.Exp, scale=0.125)
                    j = t2t - 4 * blk
                    if 0 <= j <= 3:
                        # diagonal 128x128 triangle: keep where t1 - t2 >= 0
                        nc.gpsimd.affine_select(
                            out=et[:, t2t, a : a + 128], in_=et[:, t2t, a : a + 128],
                            compare_op=ALU.is_ge, fill=0.0,
                            base=0, pattern=[[1, 128]], channel_multiplier=-1,
                        )
                ps_o = o_psum.tile([65, 512], F32, tag="o")
                for t2t in range(t2max):
                    a = max(0, 128 * (t2t - 4 * blk))
                    nc.tensor.matmul(
                        ps_o[:, a:], vaug[:, t2t, h, :], et[:, t2t, a:],
                        start=(t2t == 0), stop=(t2t == t2max - 1),
                        skip_group_check=True,
                    )
                # 1/den = exp(-ln(den)); broadcast over the 64 head dims via PE
                lnd = rpool.tile([1, 512], F32, tag="lnd")
                nc.scalar.activation(lnd, ps_o[64:65, :], AF.Ln)
                rec = rpool.tile([1, 512], BF, tag="rec")
                nc.scalar.activation(rec, lnd, AF.Exp, scale=-1.0)
                ps_rb = bc_psum.tile([128, 512], F32, tag="bc")
                nc.tensor.matmul(ps_rb[0:64, :], ones_m[:, 0:64], rec,
                                 start=True, stop=True)
                osb = spool.tile([64, 512], BF, tag="osb")
                nc.vector.tensor_copy(osb, ps_o[0:64, :])
                nc.vector.tensor_mul(oT[prow, hm, t1base : t1base + 512],
                                     osb, ps_rb[0:64, :])

        def outproj_block(nb, wo_sb, ev1_sb):
            tsl = slice(nb * 512, (nb + 1) * 512)
            ar_in = dram.tile([D, 512], BF, tag="arin")
            for mt in range(KT):
                ps = mm_psum.tile([128, 512], F32, tag="mm")
                for kt in range(2):
                    nc.tensor.matmul(
                        ps, wo_sb[:, kt, mt * 128 : (mt + 1) * 128], oT[:, kt, tsl],
                        start=(kt == 0), stop=(kt == 1),
                    )
                ob = spool.tile([128, 512], BF, tag="ob")
                evict(ps, ob, col=ev1_sb[:, mt : mt + 1], eng=RR[mt % 2])
                nc.sync.dma_start(ar_in[mt * 128 : (mt + 1) * 128, :], ob)
            ar_out = dram.tile([D, 512], BF, tag="arout")
            nc.gpsimd.collective_compute(
                "AllReduce", ALU.add, replica_groups=RG,
                ins=[ar_in.opt()], outs=[ar_out.opt()],
            )
            return ar_out

        def ffn_block(nb, w1_sb, w2_sb, b1_sb, ev2_sb):
            tsl = slice(nb * 512, (nb + 1) * 512)
            for mt in range(KT):
                ps = mm_psum.tile([128, 512], F32, tag="mm")
                for kt in range(KT):
                    nc.tensor.matmul(
                        ps, w1_sb[:, kt, mt * 128 : (mt + 1) * 128], hb[:, kt, tsl],
                        start=(kt == 0), stop=(kt == KT - 1),
                    )
                evict(ps, f1[:, mt, tsl], col=b1_sb[:, mt : mt + 1], relu=True,
                      eng=RR[mt % 2])
            ar_in = dram.tile([D, 512], BF, tag="arin")
            for mt in range(KT):
                ps = mm_psum.tile([128, 512], F32, tag="mm")
                for kt in range(KT):
                    nc.tensor.matmul(
                        ps, w2_sb[:, kt, mt * 128 : (mt + 1) * 128], f1[:, kt, tsl],
                        start=(kt == 0), stop=(kt == KT - 1),
                    )
                ob = spool.tile([128, 512], BF, tag="ob")
                evict(ps, ob, col=ev2_sb[:, mt : mt + 1], eng=RR[(mt + 1) % 2])
                nc.sync.dma_start(ar_in[mt * 128 : (mt + 1) * 128, :], ob)
            ar_out = dram.tile([D, 512], BF, tag="arout")
            nc.gpsimd.collective_compute(
                "AllReduce", ALU.add, replica_groups=RG,
                ins=[ar_in.opt()], outs=[ar_out.opt()],
            )
            return ar_out

        def load_cols(param, l, tag):
            t = bpool.tile([128, KT], F32, tag=tag)
            nc.sync.dma_start(t, param[l].rearrange("(m p) -> p m", p=128))
            return t

        # ---- layer loop; FFN AllReduces carried into the next iteration ----
        ar_ffn = [None, None]
        gb_prev = None
        for l in range(N_LAYERS):
            wqkv_sb = wq_pool.tile([128, KT, 768], BF, tag="wqkv")
            nc.sync.dma_start(wqkv_sb, _r2(wqkv[l]))
            wo_sb = wq_pool.tile([128, 2, D], BF, tag="wo")
            nc.sync.dma_start(wo_sb, _r2(wo[l]))
            w1_sb = wf_pool.tile([128, KT, FFL], BF, tag="w1")
            nc.sync.dma_start(w1_sb, _r2(w1[l]))
            w2_sb = wf_pool.tile([128, KT, D], BF, tag="w2")
            nc.sync.dma_start(w2_sb, _r2(w2[l]))
            bqk_sb = bpool.tile([128, 4], F32, tag="bqk")
            nc.sync.dma_start(bqk_sb, bqk[l].rearrange("(m p) -> p m", p=128))
            ev1_sb = load_cols(evb1, l, "ev1")
            ev2_sb = load_cols(evb2, l, "ev2")
            ga_sb = load_cols(resga, l, "ga")     # gamma2[l-1] (ones at l=0)
            gb_sb = load_cols(resgb, l, "gb")     # gamma1[l]
            b1_sb = load_cols(b1p, l, "b1")

            # LN2 of previous layer (consumes prev FFN ARs), then this layer's
            # QKV + attention + out-proj, per token block, so each AllReduce is
            # covered by the other block's compute
            ar_attn = [None, None]
            for nb in range(NB):
                if l > 0:
                    ln_block(nb, ar_ffn[nb], gb_prev)
                qkv_block(nb, wqkv_sb, bqk_sb)
                attn_block(nb)
                ar_attn[nb] = outproj_block(nb, wo_sb, ev1_sb)

            for nb in range(NB):
                ln_block(nb, ar_attn[nb], ga_sb)
                ar_ffn[nb] = ffn_block(nb, w1_sb, w2_sb, b1_sb, ev2_sb)
            gb_prev = gb_sb

        # ---- final LN2, vocab projection, per-core sum-exp ----
        bout_sb = const.tile([128, NVM], F32)
        nc.sync.dma_start(bout_sb, bout.rearrange("(m p) -> p m", p=128))
        for nb in range(NB):
            ln_block(nb, ar_ffn[nb], gb_prev)
            tsl = slice(nb * 512, (nb + 1) * 512)
            ps_acc_t = st_psum.tile([65, 512], F32, tag="st")
            ps_acc = ps_acc_t[0:1, :]
            for vm in range(NVM):
                wv_sb = wch_pool.tile([128, KT, 128], BF, tag="wch")
                nc.sync.dma_start(wv_sb, _r2(wout)[:, :, vm * 128 : (vm + 1) * 128])
                ps = mm_psum.tile([128, 512], F32, tag="mm")
                for kt in range(KT):
                    nc.tensor.matmul(
                        ps, wv_sb[:, kt, :], hb[:, kt, tsl],
                        start=(kt == 0), stop=(kt == KT - 1),
                    )
                outf = spool.tile([128, 512], F32, tag="outf")
                evict(ps, outf, col=bout_sb[:, vm : vm + 1], eng=RR[vm % 2])
                nc.sync.dma_start(out[vm * 128 : (vm + 1) * 128, tsl], outf)
                eb = spool.tile([128, 512], BF, tag="eb")
                nc.scalar.activation(eb, ps, AF.Exp, bias=bout_sb[:, vm : vm + 1])
                nc.tensor.matmul(
                    ps_acc, ones_1, eb,
                    start=(vm == 0), stop=(vm == NVM - 1), skip_group_check=True,
                )
            se_row = rpool.tile([1, 512], F32, tag="serow")
            nc.vector.tensor_copy(se_row, ps_acc)
            nc.sync.dma_start(sumexp[nb : nb + 1, :], se_row)

    nsplit = split_sync_waits(nc)
    print(f"split_sync_waits: {nsplit} NOPs inserted")
    return nc


def _bf16(a):
    return np.asarray(a, dtype=ml_dtypes.bfloat16)


def make_in_maps(x, tok_emb, pos_emb, wq, bq, wk, bk, wv, bv, wo, bo,
                 ln1_g, ln1_b, w1, b1, w2, b2, ln2_g, ln2_b, w_out, b_out):
    """Shard full inputs -> per-core input maps (with host-side folds)."""
    LE = wq.shape[0]
    f32 = np.float32
    # gamma2/beta2 of the *previous* layer (identity for layer 0)
    ga = np.concatenate([np.ones((1, D), f32), ln2_g[:-1]], axis=0)   # [L, D]
    be = np.concatenate([np.zeros((1, D), f32), ln2_b[:-1]], axis=0)  # [L, D]
    per_r = []
    for r in range(TP):
        hs = slice(HL * r, HL * (r + 1))
        # per-head weights, head-major concat, gamma-prev scaled rows
        wq_r = wq[:, hs].transpose(0, 2, 1, 3).reshape(LE, D, HL * DH)
        wk_r = wk[:, hs].transpose(0, 2, 1, 3).reshape(LE, D, HL * DH)
        wv_r = wv[:, hs].transpose(0, 2, 1, 3).reshape(LE, D, HL * DH)
        wqkv_r = np.concatenate([wq_r, wk_r, wv_r], axis=2) * ga[:, :, None]
        # bias folds: b' = b + W^T beta_prev
        bq_r = bq[:, hs].reshape(LE, -1) + np.einsum('ldm,ld->lm', wq_r, be)
        bk_r = bk[:, hs].reshape(LE, -1) + np.einsum('ldm,ld->lm', wk_r, be)
        bv_r = bv[:, hs].reshape(LE, -1) + np.einsum('ldm,ld->lm', wv_r, be)
        bqk_r = np.concatenate([bq_r, bk_r], axis=1).astype(f32)      # [L, 512]
        wo_r = wo[:, DH * HL * r : DH * HL * (r + 1), :]              # [L, 256, D]
        # eviction biases (pre-AllReduce, so /TP; plus folded V-bias through wo)
        ev1_r = (bo + be) / TP + np.einsum('lcd,lc->ld', wo_r, bv_r)
        ev2_r = (b2 + ln1_b) / TP
        fs = slice(FFL * r, FFL * (r + 1))
        w1_r = w1[:, :, fs] * ln1_g[:, :, None]
        b1_r = b1[:, fs] + np.einsum('ldm,ld->lm', w1[:, :, fs], ln1_b)
        vs = slice(VL * r, VL * (r + 1))
        wout_r = np.zeros((D, VLP), f32)
        wout_r[:, :VL] = w_out[:, vs] * ln2_g[-1][:, None]
        bout_r = np.full((VLP,), -1e30, f32)
        bout_r[:VL] = b_out[vs] + w_out[:, vs].T @ ln2_b[-1]
        per_r.append(dict(
            wqkv=_bf16(wqkv_r),
            bqk=np.ascontiguousarray(bqk_r),
            wo=_bf16(wo_r),
            evb1=np.ascontiguousarray(ev1_r, f32),
            evb2=np.ascontiguousarray(ev2_r, f32),
            resga=np.ascontiguousarray(ga, f32),
            resgb=np.ascontiguousarray(ln1_g, f32),
            w1=_bf16(w1_r),
            b1p=np.ascontiguousarray(b1_r, f32),
            w2=_bf16(w2[:, fs, :]),
            wout=_bf16(wout_r),
            bout=bout_r,
        ))
    in_maps = []
    for c in range(8):
        g, r = c // TP, c % TP
        emb = tok_emb[x[g]] + pos_emb[:S]          # [S, D]
        m = dict(per_r[r])
        m["h0T"] = _bf16(np.ascontiguousarray(emb.T))
        in_maps.append(m)
    return in_maps


_CACHED = {}


def _install_ntff_shim():
    """Provide antenv.axon_hooks.get_axon_ntff_profile_hook via ctypes on
    libaxon_pjrt.so (this container's trn_rl_repo snapshot lacks the module)."""
    import sys
    import types
    import ctypes
    import contextlib

    if "antenv.axon_hooks" in sys.modules:
        return
    try:
        lib = ctypes.CDLL("/opt/axon/libaxon_pjrt.so")
    except OSError:
        return
    lib.axon_start_nrt_profile.restype = ctypes.c_int64
    lib.axon_start_nrt_profile.argtypes = [ctypes.c_char_p, ctypes.c_size_t]
    lib.axon_stop_nrt_profile.restype = ctypes.c_int64
    lib.axon_stop_nrt_profile.argtypes = [ctypes.c_char_p]

    def get_axon_ntff_profile_hook():
        @contextlib.contextmanager
        def hook(neff_dir, trace_model_indices):
            d = str(neff_dir).encode()
            if lib.axon_start_nrt_profile(d, len(d)) != 0:
                yield
                return
            try:
                yield
            finally:
                lib.axon_stop_nrt_profile(d)

        return hook

    mod = types.ModuleType("antenv.axon_hooks")
    mod.get_axon_ntff_profile_hook = get_axon_ntff_profile_hook
    sys.modules["antenv.axon_hooks"] = mod


def kernel(**inputs):
    inputs = {k: np.asarray(v) for k, v in inputs.items()}
    if "nc" not in _CACHED:
        _CACHED["nc"] = build_program()
    nc = _CACHED["nc"]
    in_maps = make_in_maps(**inputs)
    trace = os.environ.get("BASS_GPT_TRACE", "0") == "1"
    if trace:
        _install_ntff_shim()
    res = run_bass_kernel_spmd(
        nc, in_maps, core_ids=list(range(8)), trace=trace,
    )
    if trace:
        print(f"HW exec time: {res.exec_time_ns} ns")
        _CACHED["last_result"] = res
    results = res.results
    full = np.empty((B, S, V), np.float32)
    for g in range(B):
        se = np.zeros((T,), np.float64)
        for r in range(TP):
            se += results[g * TP + r]["sumexp"].reshape(T).astype(np.float64)
        lse = np.log(se).astype(np.float32)                  # [T]
        for r in range(TP):
            c = g * TP + r
            full[g, :, VL * r : VL * (r + 1)] = (
                results[c]["out"][:VL, :] - lse[None, :]
            ).T
    return full


# revision 28
# speedup vs baseline: 1.6582x; 1.0181x over previous
"""GPT-style transformer forward on 8 Trainium2 NeuronCores.

Sharding: data-parallel over batch (2 groups of 4 cores), tensor-parallel
within each group (heads / FFN hidden / vocab columns split 4 ways).
Device activations are feature-major [feature, token] so all matmuls run
without transposes.

Schedule: per layer the token dim is split into two 512-blocks; each
block's AllReduce is issued while the other block computes (attention,
FFN, or the next layer's QKV), hiding collective latency.  LN gamma/beta
and all linear biases are folded into weights / eviction biases host-side,
so the device residual+LN is: fused residual-scale-add (gpsimd STT) ->
sum/sumsq ones-matmuls -> rsqrt -> two broadcast-apply ops.  Causal
attention computes only unmasked column ranges; diagonal tiles are masked
in-place with affine_select.  The log-softmax denominator is reduced on
device (per-core sum-exp) and the final subtract is folded into the
host-side unshard.
"""

import os
from contextlib import ExitStack

import numpy as np
import ml_dtypes

import concourse.bass as bass
import concourse.mybir as mybir
import concourse.tile as tile
from concourse.bass_utils import run_bass_kernel_spmd
from concourse.vector_clock import ScopedClock


def _drain_and_barrier(self, tick_clock, wait_clock):
    """The walrus build here encodes Drain/NoOp as TPB_CTRL with at most one
    sync-wait slot; Tile's stock tail attaches all outstanding waits to the
    Drain and fails codegen. Split the waits one-per-NOP instead."""
    nop_inst = self.nc.sync.nop(nofuse=True)
    wait_clock.add_sem_waits(nop_inst.ins, ScopedClock({None: tick_clock.global_clock}))
    si = nop_inst.ins.sync_info
    if si is not None and len(si.on_wait) > 1:
        waits = list(si.on_wait)
        nop_inst.ins.sync_info = mybir.SyncInfo(on_wait=waits[:1], on_update=list(si.on_update))
        for w in waits[1:]:
            n2 = self.nc.sync.nop(nofuse=True)
            n2.ins.sync_info = mybir.SyncInfo(on_wait=[w], on_update=[])
    self.nc.sync.drain()
    self.nc.all_engine_barrier()
    assert self.sems is not None
    popped = self.nc._tile_sem_poison_stack.pop()
    assert popped is self._sem_poison
    self.nc.clear_and_free_semaphores(list(self.sems.allocated().values()))
    self.nc.all_engine_barrier()


tile.TileContext._drain_and_barrier = _drain_and_barrier

_MAX_WAITS = 1  # this walrus build caps sync-waits per instruction


def split_sync_waits(nc):
    """Hoist excess on_wait entries onto same-engine NOPs inserted before the
    instruction (engine queues execute in program order, so semantics hold)."""
    n = 0
    for bb in nc.main_func.blocks:
        insts = bb.instructions
        new_list = []
        for inst in insts:
            si = getattr(inst, "sync_info", None)
            if si is not None and len(si.on_wait) > _MAX_WAITS:
                waits = list(si.on_wait)
                for w in waits[:-_MAX_WAITS]:
                    n += 1
                    new_list.append(mybir.InstNoOp(
                        name=f"{inst.name}-sw{n}",
                        sync_info=mybir.SyncInfo(on_wait=[w], on_update=[]),
                        bass_nofuse=True,
                        engine=inst.engine,
                    ))
                inst.sync_info = mybir.SyncInfo(
                    on_wait=waits[-_MAX_WAITS:], on_update=list(si.on_update)
                )
            new_list.append(inst)
        if len(new_list) != len(insts):
            bb.instructions[:] = new_list
    return n


# Model dims (hardcoded per problem spec)
L_FULL, H, D, V, SMAX = 8, 16, 1024, 32000, 1024
DH = D // H          # 64
FF = 4 * D           # 4096
B, S = 2, 1024
T = S                # tokens per group (one batch element per group)
TP = 4               # tensor-parallel degree within a group
HL = H // TP         # 4 local heads
FFL = FF // TP       # 1024 local FFN cols
VL = V // TP         # 8000 local vocab cols
VLP = 8064           # padded to 63*128
NVM = VLP // 128     # 63 vocab m-tiles
EPS = 1e-5
KT = D // 128        # 8 k-tiles over model dim
NB = T // 512        # 2 token blocks of 512

BF = mybir.dt.bfloat16
F32 = mybir.dt.float32
AF = mybir.ActivationFunctionType
ALU = mybir.AluOpType

RG = [[0, 1, 2, 3], [4, 5, 6, 7]]

N_LAYERS = int(os.environ.get("BASS_GPT_LAYERS", str(L_FULL)))
SKIP_FINAL = os.environ.get("BASS_GPT_SKIP_FINAL", "0") == "1"


def _r2(ap):
    """[ (kt p) n ] -> [p kt n] view of a DRAM 2-D tensor (p=128)."""
    return ap.rearrange("(kt p) n -> p kt n", p=128)


def build_program():
    nc = bass.Bass("TRN2")

    # ---- DRAM parameters (per-core shards; gamma/beta/bias folded host-side) ----
    h0T = nc.declare_dram_parameter("h0T", [D, T], BF, isOutput=False)
    wqkv = nc.declare_dram_parameter("wqkv", [N_LAYERS, D, 3 * HL * DH], BF, isOutput=False)
    bqk = nc.declare_dram_parameter("bqk", [N_LAYERS, 4 * 128], F32, isOutput=False)
    wo = nc.declare_dram_parameter("wo", [N_LAYERS, HL * DH, D], BF, isOutput=False)
    evb1 = nc.declare_dram_parameter("evb1", [N_LAYERS, D], F32, isOutput=False)
    evb2 = nc.declare_dram_parameter("evb2", [N_LAYERS, D], F32, isOutput=False)
    resga = nc.declare_dram_parameter("resga", [N_LAYERS, D], F32, isOutput=False)
    resgb = nc.declare_dram_parameter("resgb", [N_LAYERS, D], F32, isOutput=False)
    w1 = nc.declare_dram_parameter("w1", [N_LAYERS, D, FFL], BF, isOutput=False)
    b1p = nc.declare_dram_parameter("b1p", [N_LAYERS, FFL], F32, isOutput=False)
    w2 = nc.declare_dram_parameter("w2", [N_LAYERS, FFL, D], BF, isOutput=False)
    wout = nc.declare_dram_parameter("wout", [D, VLP], BF, isOutput=False)
    bout = nc.declare_dram_parameter("bout", [VLP], F32, isOutput=False)
    out = nc.declare_dram_parameter("out", [VLP, T], F32, isOutput=True)
    sumexp = nc.declare_dram_parameter("sumexp", [NB, 512], F32, isOutput=True)

    with ExitStack() as ctx:
        tc = ctx.enter_context(tile.TileContext(nc))

        const = ctx.enter_context(tc.tile_pool(name="const", bufs=1))
        hpool = ctx.enter_context(tc.tile_pool(name="hpool", bufs=1))
        apool = ctx.enter_context(tc.tile_pool(name="apool", bufs=1))
        epool = ctx.enter_context(tc.tile_pool(name="epool", bufs=2))
        s1pool = ctx.enter_context(tc.tile_pool(name="s1pool", bufs=1))
        wq_pool = ctx.enter_context(tc.tile_pool(name="wq_pool", bufs=2))
        wf_pool = ctx.enter_context(tc.tile_pool(name="wf_pool", bufs=1))
        wch_pool = ctx.enter_context(tc.tile_pool(name="wch_pool", bufs=3))
        bpool = ctx.enter_context(tc.tile_pool(name="bpool", bufs=2))
        spool = ctx.enter_context(tc.tile_pool(name="spool", bufs=2))
        rpool = ctx.enter_context(tc.tile_pool(name="rpool", bufs=2))
        fpool = ctx.enter_context(tc.tile_pool(name="fpool", bufs=1))

        mm_psum = ctx.enter_context(tc.tile_pool(name="mm_psum", bufs=3, space="PSUM"))
        o_psum = ctx.enter_context(tc.tile_pool(name="o_psum", bufs=2, space="PSUM"))
        st_psum = ctx.enter_context(tc.tile_pool(name="st_psum", bufs=2, space="PSUM"))
        bc_psum = ctx.enter_context(tc.tile_pool(name="bc_psum", bufs=1, space="PSUM"))

        dram = ctx.enter_context(tc.tile_pool(name="dram", bufs=3, space="DRAM"))

        # ---- constants ----
        ones_d = const.tile([128, 1], BF)       # partition-sum lhsT, scaled 1/D (LN stats)
        nc.vector.memset(ones_d, 1.0 / D)
        ones_1 = const.tile([128, 1], BF)       # partition-sum lhsT (softmax denominator)
        nc.vector.memset(ones_1, 1.0)
        ones_m = const.tile([1, 128], BF)       # broadcast lhsT (K=1, M=128)
        nc.vector.memset(ones_m, 1.0)
        negones_m = const.tile([1, 128], BF)    # negated broadcast lhsT
        nc.vector.memset(negones_m, -1.0)
        eps_sb = const.tile([1, 1], F32)
        nc.vector.memset(eps_sb, float(EPS))

        # ---- persistent activation state ----
        hb = hpool.tile([128, KT, T], BF)       # residual stream (feature-major, pre-gamma/beta)
        qk_sb = apool.tile([128, 2, 2, T], BF)  # [part, q/k, head-pair, t]
        vaug = apool.tile([128, KT, HL, 65], BF)  # token-major V + ones col
        oT = apool.tile([128, 2, T], BF)        # attn head outputs (feature-major, normalized)
        f1 = fpool.tile([128, KT, T], BF)       # FFN hidden (local)

        nc.sync.dma_start(hb, _r2(h0T))         # h0 straight into the residual stream
        nc.vector.memset(vaug[:, :, :, 64:65], 1.0)

        def ln_block(nb, ar_out, gcol):
            """hb[:, :, tsl] <- normalize(hb * gcol + AR result) (token block nb)."""
            tsl = slice(nb * 512, (nb + 1) * 512)
            arb = s1pool.tile([128, KT, 512], BF, tag="arb")
            nc.sync.dma_start(arb, _r2(ar_out))
            xb = s1pool.tile([128, KT, 512], BF, tag="xb")
            ps_st = st_psum.tile([65, 512], F32, tag="st")
            for kt in range(KT):
                nc.vector.scalar_tensor_tensor(
                    out=xb[:, kt, :], in0=hb[:, kt, tsl],
                    scalar=gcol[:, kt : kt + 1], in1=arb[:, kt, :],
                    op0=ALU.mult, op1=ALU.add,
                )
                xsq = spool.tile([128, 512], BF, tag="xsq")
                nc.scalar.activation(xsq, xb[:, kt, :], AF.Square)
                nc.tensor.matmul(ps_st[0:1, :], ones_d, xb[:, kt, :],
                                 start=(kt == 0), stop=(kt == KT - 1),
                                 skip_group_check=True)
                nc.tensor.matmul(ps_st[64:65, :], ones_d, xsq,
                                 start=(kt == 0), stop=(kt == KT - 1),
                                 skip_group_check=True)
            # u = m2 - mu^2 ; a = rsqrt(u + eps) = exp(-0.5 ln(u + eps))
            negs1r = rpool.tile([1, 512], BF, tag="negs1r")
            nc.vector.tensor_scalar(out=negs1r, in0=ps_st[0:1, :], scalar1=-1.0,
                                    scalar2=None, op0=ALU.mult)
            t0n = rpool.tile([1, 512], F32, tag="t0n")
            nc.vector.tensor_mul(t0n, negs1r, ps_st[0:1, :])       # -mu^2
            u = rpool.tile([1, 512], F32, tag="u")
            nc.vector.tensor_add(u, t0n, ps_st[64:65, :])
            lnr = rpool.tile([1, 512], F32, tag="lnr")
            nc.scalar.activation(lnr, u, AF.Ln, bias=eps_sb[0:1, 0:1])
            a_row = rpool.tile([1, 512], BF, tag="arow")
            nc.scalar.activation(a_row, lnr, AF.Exp, scale=-0.5)
            # broadcast -mu and a across partitions via K=1 matmuls, stage in SBUF
            ps_nm = bc_psum.tile([128, 512], F32, tag="bc")
            nc.tensor.matmul(ps_nm, ones_m, negs1r, start=True, stop=True)
            nmb = spool.tile([128, 512], BF, tag="nmb")
            nc.scalar.activation(nmb, ps_nm, AF.Identity)
            ps_ab = bc_psum.tile([128, 512], F32, tag="bc")
            nc.tensor.matmul(ps_ab, ones_m, a_row, start=True, stop=True)
            ab = spool.tile([128, 512], BF, tag="ab")
            nc.scalar.activation(ab, ps_ab, AF.Identity)
            for kt in range(KT):
                tt = spool.tile([128, 512], BF, tag="lnt")
                # gpsimd TT runs ~2.7x slower than vector bf16; give it 1/4
                e_add = nc.gpsimd if kt % 4 == 3 else nc.vector
                e_mul = nc.gpsimd if kt % 4 == 1 else nc.vector
                e_add.tensor_add(tt, xb[:, kt, :], nmb)
                e_mul.tensor_mul(hb[:, kt, tsl], tt, ab)

        def evict(ps, out_ap, col=None, relu=False, eng="scalar"):
            """PSUM -> SBUF eviction with optional per-partition bias / relu.
            (gpsimd cannot read PSUM, so only scalar/vector qualify.)"""
            if eng == "scalar":
                nc.scalar.activation(out_ap, ps, AF.Relu if relu else AF.Identity,
                                     bias=col if col is not None else 0.0)
            else:
                e = nc.vector
                if relu:
                    e.tensor_scalar(out=out_ap, in0=ps,
                                    scalar1=col if col is not None else 0.0,
                                    scalar2=0.0, op0=ALU.add, op1=ALU.max)
                elif col is not None:
                    e.tensor_scalar(out=out_ap, in0=ps, scalar1=col, scalar2=None,
                                    op0=ALU.add)
                else:
                    e.tensor_copy(out_ap, ps)

        RR = ("scalar", "vector")

        def qkv_block(nb, wqkv_sb, bqk_sb):
            tsl = slice(nb * 512, (nb + 1) * 512)
            for io in range(2):        # 0=q, 1=k  (feature-major out)
                for mt in range(2):    # head pair
                    mcol = (io * 2 + mt) * 128
                    ps = mm_psum.tile([128, 512], F32, tag="mm")
                    for kt in range(KT):
                        nc.tensor.matmul(
                            ps, wqkv_sb[:, kt, mcol : mcol + 128], hb[:, kt, tsl],
                            start=(kt == 0), stop=(kt == KT - 1),
                        )
                    evict(ps, qk_sb[:, io, mt, tsl],
                          col=bqk_sb[:, io * 2 + mt : io * 2 + mt + 1],
                          eng=RR[(io * 2 + mt) % 2])
            for tm in range(nb * 4, nb * 4 + 4):   # v, token-major
                ps = mm_psum.tile([128, 512], F32, tag="mm")
                for kt in range(KT):
                    nc.tensor.matmul(
                        ps[:, 0:256], hb[:, kt, tm * 128 : (tm + 1) * 128],
                        wqkv_sb[:, kt, 512:768],
                        start=(kt == 0), stop=(kt == KT - 1),
                    )
                evict(ps[:, 0:256].rearrange("p (h e) -> p h e", h=HL),
                      vaug[:, tm, :, 0:64], eng=RR[tm % 2])

        def attn_block(blk):
            t1base = blk * 512
            t2max = 4 * (blk + 1)
            for h in range(HL):
                prow = slice(64 * (h % 2), 64 * (h % 2) + 64)
                hm = h // 2
                et = epool.tile([128, 8, 512], BF, tag="eT")
                for t2t in range(t2max):
                    a = max(0, 128 * (t2t - 4 * blk))
                    ps = mm_psum.tile([128, 512], F32, tag="mm")
                    nc.tensor.matmul(
                        ps[:, a:],
                        qk_sb[prow, 1, hm, t2t * 128 : (t2t + 1) * 128],
                        qk_sb[prow, 0, hm, t1base + a : t1base + 512],
                        start=True, stop=True,
                    )
                    nc.scalar.activation(et[:, t2t, a:], ps[:, a:], AF.Exp, scale=0.125)
                    j = t2t - 4 * blk
                    if 0 <= j <= 3:
                        # diagonal 128x128 triangle: keep where t1 - t2 >= 0
                        nc.gpsimd.affine_select(
                            out=et[:, t2t, a : a + 128], in_=et[:, t2t, a : a + 128],
                            compare_op=ALU.is_ge, fill=0.0,
                            base=0, pattern=[[1, 128]], channel_multiplier=-1,
                        )
                ps_o = o_psum.tile([65, 512], F32, tag="o")
                for t2t in range(t2max):
                    a = max(0, 128 * (t2t - 4 * blk))
                    nc.tensor.matmul(
                        ps_o[:, a:], vaug[:, t2t, h, :], et[:, t2t, a:],
                        start=(t2t == 0), stop=(t2t == t2max - 1),
                        skip_group_check=True,
                    )
                # 1/den = exp(-ln(den)); broadcast over the 64 head dims via PE
                lnd = rpool.tile([1, 512], F32, tag="lnd")
                nc.scalar.activation(lnd, ps_o[64:65, :], AF.Ln)
                rec = rpool.tile([1, 512], BF, tag="rec")
                nc.scalar.activation(rec, lnd, AF.Exp, scale=-1.0)
                ps_rb = bc_psum.tile([128, 512], F32, tag="bc")
                nc.tensor.matmul(ps_rb[0:64, :], ones_m[:, 0:64], rec,
                                 start=True, stop=True)
                osb = spool.tile([64, 512], BF, tag="osb")
                nc.vector.tensor_copy(osb, ps_o[0:64, :])
                nc.vector.tensor_mul(oT[prow, hm, t1base : t1base + 512],
                                     osb, ps_rb[0:64, :])

        def outproj_block(nb, wo_sb, ev1_sb):
            tsl = slice(nb * 512, (nb + 1) * 512)
            ar_in = dram.tile([D, 512], BF, tag="arin")
            for mt in range(KT):
                ps = mm_psum.tile([128, 512], F32, tag="mm")
                for kt in range(2):
                    nc.tensor.matmul(
                        ps, wo_sb[:, kt, mt * 128 : (mt + 1) * 128], oT[:, kt, tsl],
                        start=(kt == 0), stop=(kt == 1),
                    )
                ob = spool.tile([128, 512], BF, tag="ob")
                evict(ps, ob, col=ev1_sb[:, mt : mt + 1], eng=RR[mt % 2])
                nc.sync.dma_start(ar_in[mt * 128 : (mt + 1) * 128, :], ob)
            ar_out = dram.tile([D, 512], BF, tag="arout")
            nc.gpsimd.collective_compute(
                "AllReduce", ALU.add, replica_groups=RG,
                ins=[ar_in.opt()], outs=[ar_out.opt()],
            )
            return ar_out

        def ffn_block(nb, w1_sb, w2_sb, b1_sb, ev2_sb):
            tsl = slice(nb * 512, (nb + 1) * 512)
            for mt in range(KT):
                ps = mm_psum.tile([128, 512], F32, tag="mm")
                for kt in range(KT):
                    nc.tensor.matmul(
                        ps, w1_sb[:, kt, mt * 128 : (mt + 1) * 128], hb[:, kt, tsl],
                        start=(kt == 0), stop=(kt == KT - 1),
                    )
                evict(ps, f1[:, mt, tsl], col=b1_sb[:, mt : mt + 1], relu=True,
                      eng=RR[mt % 2])
            ar_in = dram.tile([D, 512], BF, tag="arin")
            for mt in range(KT):
                ps = mm_psum.tile([128, 512], F32, tag="mm")
                for kt in range(KT):
                    nc.tensor.matmul(
                        ps, w2_sb[:, kt, mt * 128 : (mt + 1) * 128], f1[:, kt, tsl],
                        start=(kt == 0), stop=(kt == KT - 1),
                    )
                ob = spool.tile([128, 512], BF, tag="ob")
                evict(ps, ob, col=ev2_sb[:, mt : mt + 1], eng=RR[(mt + 1) % 2])
                nc.sync.dma_start(ar_in[mt * 128 : (mt + 1) * 128, :], ob)
            ar_out = dram.tile([D, 512], BF, tag="arout")
            nc.gpsimd.collective_compute(
                "AllReduce", ALU.add, replica_groups=RG,
                ins=[ar_in.opt()], outs=[ar_out.opt()],
            )
            return ar_out

        def load_cols(param, l, tag):
            t = bpool.tile([128, KT], F32, tag=tag)
            nc.sync.dma_start(t, param[l].rearrange("(m p) -> p m", p=128))
            return t

        # ---- layer loop; FFN AllReduces carried into the next iteration ----
        ar_ffn = [None, None]
        gb_prev = None
        for l in range(N_LAYERS):
            wqkv_sb = wq_pool.tile([128, KT, 768], BF, tag="wqkv")
            nc.sync.dma_start(wqkv_sb, _r2(wqkv[l]))
            wo_sb = wq_pool.tile([128, 2, D], BF, tag="wo")
            nc.sync.dma_start(wo_sb, _r2(wo[l]))
            w1_sb = wf_pool.tile([128, KT, FFL], BF, tag="w1")
            nc.sync.dma_start(w1_sb, _r2(w1[l]))
            w2_sb = wf_pool.tile([128, KT, D], BF, tag="w2")
            nc.sync.dma_start(w2_sb, _r2(w2[l]))
            bqk_sb = bpool.tile([128, 4], F32, tag="bqk")
            nc.sync.dma_start(bqk_sb, bqk[l].rearrange("(m p) -> p m", p=128))
            ev1_sb = load_cols(evb1, l, "ev1")
            ev2_sb = load_cols(evb2, l, "ev2")
            ga_sb = load_cols(resga, l, "ga")     # gamma2[l-1] (ones at l=0)
            gb_sb = load_cols(resgb, l, "gb")     # gamma1[l]
            b1_sb = load_cols(b1p, l, "b1")

            # LN2 of previous layer (consumes prev FFN ARs), then this layer's
            # QKV + attention + out-proj, per token block, so each AllReduce is
            # covered by the other block's compute
            ar_attn = [None, None]
            for nb in range(NB):
                if l > 0:
                    ln_block(nb, ar_ffn[nb], gb_prev)
                qkv_block(nb, wqkv_sb, bqk_sb)
                attn_block(nb)
                ar_attn[nb] = outproj_block(nb, wo_sb, ev1_sb)

            for nb in range(NB):
                ln_block(nb, ar_attn[nb], ga_sb)
                ar_ffn[nb] = ffn_block(nb, w1_sb, w2_sb, b1_sb, ev2_sb)
            gb_prev = gb_sb

        # ---- final LN2, vocab projection, per-core sum-exp ----
        bout_sb = const.tile([128, NVM], F32)
        nc.sync.dma_start(bout_sb, bout.rearrange("(m p) -> p m", p=128))
        for nb in range(NB):
            ln_block(nb, ar_ffn[nb], gb_prev)
            tsl = slice(nb * 512, (nb + 1) * 512)
            ps_acc_t = st_psum.tile([65, 512], F32, tag="st")
            ps_acc = ps_acc_t[0:1, :]
            for vm in range(NVM):
                wv_sb = wch_pool.tile([128, KT, 128], BF, tag="wch")
                nc.sync.dma_start(wv_sb, _r2(wout)[:, :, vm * 128 : (vm + 1) * 128])
                ps = mm_psum.tile([128, 512], F32, tag="mm")
                for kt in range(KT):
                    nc.tensor.matmul(
                        ps, wv_sb[:, kt, :], hb[:, kt, tsl],
                        start=(kt == 0), stop=(kt == KT - 1),
                    )
                outf = spool.tile([128, 512], F32, tag="outf")
                evict(ps, outf, col=bout_sb[:, vm : vm + 1], eng=RR[vm % 2])
                nc.sync.dma_start(out[vm * 128 : (vm + 1) * 128, tsl], outf)
                eb = spool.tile([128, 512], BF, tag="eb")
                nc.scalar.activation(eb, ps, AF.Exp, bias=bout_sb[:, vm : vm + 1])
                nc.tensor.matmul(
                    ps_acc, ones_1, eb,
                    start=(vm == 0), stop=(vm == NVM - 1), skip_group_check=True,
                )
            se_row = rpool.tile([1, 512], F32, tag="serow")
            nc.vector.tensor_copy(se_row, ps_acc)
            nc.sync.dma_start(sumexp[nb : nb + 1, :], se_row)

    nsplit = split_sync_waits(nc)
    print(f"split_sync_waits: {nsplit} NOPs inserted")
    return nc


def _bf16(a):
    return np.asarray(a, dtype=ml_dtypes.bfloat16)


def make_in_maps(x, tok_emb, pos_emb, wq, bq, wk, bk, wv, bv, wo, bo,
                 ln1_g, ln1_b, w1, b1, w2, b2, ln2_g, ln2_b, w_out, b_out):
    """Shard full inputs -> per-core input maps (with host-side folds)."""
    LE = wq.shape[0]
    f32 = np.float32
    # gamma2/beta2 of the *previous* layer (identity for layer 0)
    ga = np.concatenate([np.ones((1, D), f32), ln2_g[:-1]], axis=0)   # [L, D]
    be = np.concatenate([np.zeros((1, D), f32), ln2_b[:-1]], axis=0)  # [L, D]
    per_r = []
    for r in range(TP):
        hs = slice(HL * r, HL * (r + 1))
        # per-head weights, head-major concat, gamma-prev scaled rows
        wq_r = wq[:, hs].transpose(0, 2, 1, 3).reshape(LE, D, HL * DH)
        wk_r = wk[:, hs].transpose(0, 2, 1, 3).reshape(LE, D, HL * DH)
        wv_r = wv[:, hs].transpose(0, 2, 1, 3).reshape(LE, D, HL * DH)
        wqkv_r = np.concatenate([wq_r, wk_r, wv_r], axis=2) * ga[:, :, None]
        # bias folds: b' = b + W^T beta_prev
        bq_r = bq[:, hs].reshape(LE, -1) + np.einsum('ldm,ld->lm', wq_r, be)
        bk_r = bk[:, hs].reshape(LE, -1) + np.einsum('ldm,ld->lm', wk_r, be)
        bv_r = bv[:, hs].reshape(LE, -1) + np.einsum('ldm,ld->lm', wv_r, be)
        bqk_r = np.concatenate([bq_r, bk_r], axis=1).astype(f32)      # [L, 512]
        wo_r = wo[:, DH * HL * r : DH * HL * (r + 1), :]              # [L, 256, D]
        # eviction biases (pre-AllReduce, so /TP; plus folded V-bias through wo)
        ev1_r = (bo + be) / TP + np.einsum('lcd,lc->ld', wo_r, bv_r)
        ev2_r = (b2 + ln1_b) / TP
        fs = slice(FFL * r, FFL * (r + 1))
        w1_r = w1[:, :, fs] * ln1_g[:, :, None]
        b1_r = b1[:, fs] + np.einsum('ldm,ld->lm', w1[:, :, fs], ln1_b)
        vs = slice(VL * r, VL * (r + 1))
        wout_r = np.zeros((D, VLP), f32)
        wout_r[:, :VL] = w_out[:, vs] * ln2_g[-1][:, None]
        bout_r = np.full((VLP,), -1e30, f32)
        bout_r[:VL] = b_out[vs] + w_out[:, vs].T @ ln2_b[-1]
        per_r.append(dict(
            wqkv=_bf16(wqkv_r),
            bqk=np.ascontiguousarray(bqk_r),
            wo=_bf16(wo_r),
            evb1=np.ascontiguousarray(ev1_r, f32),
            evb2=np.ascontiguousarray(ev2_r, f32),
            resga=np.ascontiguousarray(ga, f32),
            resgb=np.ascontiguousarray(ln1_g, f32),
            w1=_bf16(w1_r),
            b1p=np.ascontiguousarray(b1_r, f32),
            w2=_bf16(w2[:, fs, :]),
            wout=_bf16(wout_r),
            bout=bout_r,
        ))
    in_maps = []
    for c in range(8):
        g, r = c // TP, c % TP
        emb = tok_emb[x[g]] + pos_emb[:S]          # [S, D]
        m = dict(per_r[r])
        m["h0T"] = _bf16(np.ascontiguousarray(emb.T))
        in_maps.append(m)
    return in_maps


_CACHED = {}


def _install_ntff_shim():
    """Provide antenv.axon_hooks.get_axon_ntff_profile_hook via ctypes on
    libaxon_pjrt.so (this container's trn_rl_repo snapshot lacks the module)."""
    import sys
    import types
    import ctypes
    import contextlib

    if "antenv.axon_hooks" in sys.modules:
        return
    try:
        lib = ctypes.CDLL("/opt/axon/libaxon_pjrt.so")
    except OSError:
        return
    lib.axon_start_nrt_profile.restype = ctypes.c_int64
    lib.axon_start_nrt_profile.argtypes = [ctypes.c_char_p, ctypes.c_size_t]
    lib.axon_stop_nrt_profile.restype = ctypes.c_int64
    lib.axon_stop_nrt_profile.argtypes = [ctypes.c_char_p]

    def get_axon_ntff_profile_hook():
        @contextlib.contextmanager
        def hook(neff_dir, trace_model_indices):
            d = str(neff_dir).encode()
            if lib.axon_start_nrt_profile(d, len(d)) != 0:
                yield
                return
            try:
                yield
            finally:
                lib.axon_stop_nrt_profile(d)

        return hook

    mod = types.ModuleType("antenv.axon_hooks")
    mod.get_axon_ntff_profile_hook = get_axon_ntff_profile_hook
    sys.modules["antenv.axon_hooks"] = mod


def kernel(**inputs):
    inputs = {k: np.asarray(v) for k, v in inputs.items()}
    if "nc" not in _CACHED:
        _CACHED["nc"] = build_program()
    nc = _CACHED["nc"]
    in_maps = make_in_maps(**inputs)
    trace = os.environ.get("BASS_GPT_TRACE", "0") == "1"
    if trace:
        _install_ntff_shim()
    res = run_bass_kernel_spmd(
        nc, in_maps, core_ids=list(range(8)), trace=trace,
    )
    if trace:
        print(f"HW exec time: {res.exec_time_ns} ns")
        _CACHED["last_result"] = res
    results = res.results
    full = np.empty((B, S, V), np.float32)
    for g in range(B):
        se = np.zeros((T,), np.float64)
        for r in range(TP):
            se += results[g * TP + r]["sumexp"].reshape(T).astype(np.float64)
        lse = np.log(se).astype(np.float32)                  # [T]
        for r in range(TP):
            c = g * TP + r
            full[g, :, VL * r : VL * (r + 1)] = (
                results[c]["out"][:VL, :] - lse[None, :]
            ).T
    return full
